# revision 1
# baseline (speedup 1.0000x reference)
"""Trainium2 Bass kernel for DeepConvGraphEncoderDownstream.

Model (per reference):
  4-layer GCN (shared dense 24x24 graph operator) applied per (batch, timestep)
  frame -> node-mean -> per sliding window (W=32, stride 2, 113 windows):
  BiLSTM(H=256) -> concat(h_fwd[-1], h_bwd[0]) @ Wfc + bfc.

Key algebraic restructurings:
  * gcn_norm folded into one dense Ahat[24,24] on host.
  * GCN runs ONCE over all 256 timesteps (the reference recomputes it ~14x
    across overlapping windows).
  * backward LSTM: only hb[:, 0] is used => exactly ONE step, no recurrence.
  * forward LSTM: all 113 windows batched into one 904-row recurrence per
    core; input transforms U precomputed from node-mean features.

Sharding: data-parallel over batch, 8 batches/core on 8 cores; output
slices are independent (no collectives).

Layouts (per core, per chunk = one local batch = 256 timesteps padded to
260 = 52 blocks * 5):
  A-layout [c_part, free=(gb:52, blk:128)], blk = n*5+g5 (120:128 pad),
           timestep t = 5*gb + g5.
  B-layout [blk partitions = 128, free=(gb, cblock:128 or 256)]
  A->B / B->A are single XBAR DMA-transpose instructions per c-block:
  HW semantics out[p, b, c] = in[c, b*128 + p].
  Node mixing = matmul with zero-padded stationary kron(Ahat^T, I5) [128,128].
"""

import os
import sys
import numpy as np

try:
    import concourse.bass as bass
except ImportError:
    sys.path.insert(0, "/opt/trn_rl_repo")
    import concourse.bass as bass

import concourse.bacc as bacc
import concourse.tile as tile
from concourse import mybir
from concourse import bass_utils

F16 = mybir.dt.float16
F32 = mybir.dt.float32
AF = mybir.ActivationFunctionType
ALU = mybir.AluOpType

B, T, N, FIN = 64, 256, 24, 6
H, EMB = 256, 128
WIN = 32
NW = (T - WIN) // 2 + 1               # 113
NCORES = 8
BL = B // NCORES                      # 8
G5 = 5
GBLK = 52                             # ceil(260/5): 52*5 = 260 t-slots
TP = GBLK * G5                        # 260 padded timesteps
NCH = BL
ROWS = BL * NW                        # 904
HROWS = ROWS // 2                     # 452
CH_FREE = GBLK * 128                  # 6656 A-layout free per chunk
FTOT = BL * TP                        # 2080 F columns

_CACHE = {}


def _kernel_body(tc, io):
    nc = tc.nc
    from contextlib import ExitStack
    ctx = ExitStack()

    cons = ctx.enter_context(tc.tile_pool(name="cons", bufs=1))
    fpool = ctx.enter_context(tc.tile_pool(name="fpool", bufs=1))

    def load_const(name, shape, dt=F16):
        t = cons.tile(shape, dt, name=name)
        nc.sync.dma_start(t[:], io[name][:])
        return t

    mixM = load_const("mixM", [128, 128])
    w1 = load_const("w1", [FIN, 128])
    w2 = load_const("w2", [64, 128])
    w3 = load_const("w3", [128, 256])
    b1 = load_const("b1", [128, 1], F32)
    b2 = load_const("b2", [128, 1], F32)
    b3 = load_const("b3", [128, 2], F32)
    b4 = load_const("b4", [128, 2], F32)
    ident = load_const("ident", [128, 128])
    w4k = []
    for kt in range(2):
        t = cons.tile([128, 256], F16, name=f"w4k{kt}")
        nc.sync.dma_start(t[:], io["w4"][kt * 128:(kt + 1) * 128, :])
        w4k.append(t)

    def load_ktiles(name):
        ts = []
        for kt in range(2):
            t = cons.tile([128, 1024], F16, name=f"{name}{kt}")
            nc.sync.dma_start(t[:], io[name][kt * 128:(kt + 1) * 128, :])
            ts.append(t)
        return ts

    lxf = load_ktiles("lxf")
    lhf = load_ktiles("lhf")
    lxb = load_ktiles("lxb")
    bgf = load_const("bgf", [128, 8], F32)
    bgb = load_const("bgb", [128, 8], F32)
    wfct = []
    for qt in range(4):
        t = cons.tile([128, 128], F16, name=f"wfct{qt}")
        nc.sync.dma_start(t[:], io["wfc"][qt * 128:(qt + 1) * 128, :])
        wfct.append(t)
    bfc = load_const("bfc", [128, 1], F32)

    F0 = fpool.tile([128, FTOT], F16, name="F0")
    F1 = fpool.tile([128, FTOT], F16, name="F1")
    Fts = [F0, F1]

    # ================= Phase 1: GCN =================
    with tc.tile_pool(name="gcnS", bufs=4) as gpS, \
         tc.tile_pool(name="gcnA", bufs=4) as gpA, \
         tc.tile_pool(name="gcnB", bufs=3) as gpB, \
         tc.tile_pool(name="gcnBig", bufs=1) as gpBig, \
         tc.tile_pool(name="gps", bufs=3, space="PSUM") as ps_t, \
         tc.tile_pool(name="gpsm", bufs=2, space="PSUM") as ps_m:

        def mix(src_b, n_free, dsts, split_c=False):
            """node-mix src_b [128, n_free] via mixM -> fp16 dsts
            (one tile, or two c-half tiles when split_c)."""
            for fc in range(n_free // 512):
                ps = ps_m.tile([128, 512], F32, tag="mps", name="mps")
                nc.tensor.matmul(ps[:], mixM[:],
                                 src_b[:, fc * 512:(fc + 1) * 512],
                                 start=True, stop=True)
                if not split_c:
                    nc.vector.tensor_copy(dsts[0][:, fc * 512:(fc + 1) * 512],
                                          ps[:])
                else:
                    # free = (gb, 256c): 512-chunk = 2 gb x 256 c
                    psv = ps[:].rearrange("p (g2 c) -> p g2 c", g2=2)
                    for hf in range(2):
                        dv = dsts[hf][:].rearrange("p (gb c) -> p gb c",
                                                   c=128)
                        nc.vector.tensor_copy(
                            dv[:, fc * 2:fc * 2 + 2, :],
                            psv[:, :, hf * 128:(hf + 1) * 128])

        # transform free chunking over CH_FREE=6656: 6x1024 + 1x512
        TCH = [(i * 1024, 1024) for i in range(6)] + [(6144, 512)]

        def transform(rhs_tiles, wslices, bias, cout, out_tiles, act_engine):
            nkt = len(rhs_tiles)
            for mt in range(max(cout // 128, 1)):
                for f0, fw in TCH:
                    mp = min(cout, 128)
                    ps = ps_t.tile([mp, 1024], F32, tag="tps", name="tps")
                    for fc in range(fw // 512):
                        g0 = f0 + fc * 512
                        for kt in range(nkt):
                            w = wslices[kt]
                            wap = w[:, mt * 128:(mt + 1) * 128] if cout > 128 \
                                else w
                            nc.tensor.matmul(ps[:, fc * 512:(fc + 1) * 512],
                                             wap, rhs_tiles[kt][:, g0:g0 + 512],
                                             start=(kt == 0),
                                             stop=(kt == nkt - 1))
                    dst = out_tiles[mt][:, f0:f0 + fw]
                    src = ps[:, 0:fw]
                    if act_engine == "act":
                        nc.scalar.activation(dst, src, AF.Relu,
                                             bias=bias[:, mt:mt + 1], scale=1.0)
                    else:
                        nc.vector.tensor_scalar(dst, src, bias[:, mt:mt + 1],
                                                0.0, ALU.add, ALU.max)

        for k in range(NCH):
            xb0 = gpS.tile([128, GBLK * 128], F16, tag="xsm", name="xb0")
            nc.sync.dma_start(xb0[:], io["x0B"][k])

            # --- L1: mix@6 -> transform 6->64 (output rows 64:128 zeroed
            # via zero-padded W1/b1 so downstream tiles are fully defined)
            y1b = gpB.tile([128, GBLK * 128], F16, tag="cB", name="y1b")
            mix(xb0, GBLK * 128, [y1b])
            y1a = gpA.tile([128, CH_FREE], F16, tag="cA", name="y1a")
            nc.sync.dma_start(y1a[:].rearrange("c (gb p) -> c gb p", p=128),
                              y1b[:], transpose=True)
            x1a = gpA.tile([128, CH_FREE], F16, tag="cA", name="x1a")
            transform([y1a[0:FIN]], [w1[:]], b1, 128, [x1a], "act")

            # --- L2: mix@64 -> transform 64->128
            x2b = gpB.tile([128, GBLK * 128], F16, tag="cB", name="x2b")
            nc.sync.dma_start(x2b[:].rearrange("p (gb c) -> p gb c", c=128),
                              x1a[:], transpose=True)
            y2b = gpB.tile([128, GBLK * 128], F16, tag="cB", name="y2b")
            mix(x2b, GBLK * 128, [y2b])
            y2a = gpA.tile([128, CH_FREE], F16, tag="cA", name="y2a")
            nc.sync.dma_start(y2a[:].rearrange("c (gb p) -> c gb p", p=128),
                              y2b[:], transpose=True)
            x2a = gpA.tile([128, CH_FREE], F16, tag="cA", name="x2a")
            transform([y2a[0:64]], [w2[:]], b2, 128, [x2a], "vec")

            # --- L3: mix@128 -> transform 128->256
            x3b = gpB.tile([128, GBLK * 128], F16, tag="cB", name="x3b")
            nc.sync.dma_start(x3b[:].rearrange("p (gb c) -> p gb c", c=128),
                              x2a[:], transpose=True)
            y3b = gpB.tile([128, GBLK * 128], F16, tag="cB", name="y3b")
            mix(x3b, GBLK * 128, [y3b])
            y3a = gpA.tile([128, CH_FREE], F16, tag="cA", name="y3a")
            nc.sync.dma_start(y3a[:].rearrange("c (gb p) -> c gb p", p=128),
                              y3b[:], transpose=True)
            x3a0 = gpA.tile([128, CH_FREE], F16, tag="cA", name="x3a0")
            x3a1 = gpA.tile([128, CH_FREE], F16, tag="cA", name="x3a1")
            transform([y3a], [w3[:]], b3, 256, [x3a0, x3a1], "act")

            # --- L4: mix@256 -> transform 256->256
            x4b = gpBig.tile([128, GBLK * 256], F16, tag="big", name="x4b")
            x4bv = x4b[:].rearrange("p (gb c) -> p gb c", c=256)
            nc.sync.dma_start(x4bv[:, :, 0:128], x3a0[:], transpose=True)
            nc.sync.dma_start(x4bv[:, :, 128:256], x3a1[:], transpose=True)
            ylo = gpB.tile([128, GBLK * 128], F16, tag="cB", name="ylo")
            yhi = gpB.tile([128, GBLK * 128], F16, tag="cB", name="yhi")
            mix(x4b, GBLK * 256, [ylo, yhi], split_c=True)
            y4a0 = gpA.tile([128, CH_FREE], F16, tag="cA", name="y4a0")
            y4a1 = gpA.tile([128, CH_FREE], F16, tag="cA", name="y4a1")
            nc.sync.dma_start(y4a0[:].rearrange("c (gb p) -> c gb p", p=128),
                              ylo[:], transpose=True)
            nc.sync.dma_start(y4a1[:].rearrange("c (gb p) -> c gb p", p=128),
                              yhi[:], transpose=True)
            x4a0 = gpA.tile([128, CH_FREE], F16, tag="cA", name="x4a0")
            x4a1 = gpA.tile([128, CH_FREE], F16, tag="cA", name="x4a1")
            transform([y4a0, y4a1], [w4k[0][:], w4k[1][:]], b4, 256,
                      [x4a0, x4a1], "vec")

            # node-sum into F: F[:, k*TP + t], t = 5*gb+g5
            for ct, xt in enumerate((x4a0, x4a1)):
                xv = xt[:].rearrange("p (gb blk) -> p gb blk", blk=128)
                dstv = Fts[ct][:, k * TP:(k + 1) * TP].rearrange(
                    "p (gb g5) -> p gb g5", g5=G5)
                for n in range(N):
                    if n == 0:
                        nc.vector.tensor_copy(dstv,
                                              xv[:, :, n * G5:(n + 1) * G5])
                    else:
                        nc.vector.tensor_tensor(dstv, dstv,
                                                xv[:, :, n * G5:(n + 1) * G5],
                                                ALU.add)

    # ================= Phase 2: U = F @ (Wih_f/24)^T =================
    upool = ctx.enter_context(tc.tile_pool(name="upool", bufs=1))
    UCH = [(i * 1024, 1024) for i in range(2)] + [(2048, 32)]
    Umt = []
    with tc.tile_pool(name="ups", bufs=3, space="PSUM") as ps_u:
        for mt in range(8):
            u = upool.tile([128, FTOT], F16, name=f"U{mt}")
            for f0, fw in UCH:
                ps = ps_u.tile([128, 1024], F32, tag="ups", name="ups")
                for fc in range(max(fw // 512, 1)):
                    g0 = f0 + fc * 512
                    gw = min(512, f0 + fw - g0)
                    for kt in range(2):
                        nc.tensor.matmul(ps[:, fc * 512:fc * 512 + gw],
                                         lxf[kt][:, mt * 128:(mt + 1) * 128],
                                         Fts[kt][:, g0:g0 + gw],
                                         start=(kt == 0), stop=(kt == 1))
                dst = u[:, f0:f0 + fw]
                src = ps[:, 0:fw]
                if mt % 2:
                    nc.scalar.copy(dst, src)
                else:
                    nc.vector.tensor_copy(dst, src)
            Umt.append(u)

    # ================= Phase 3: forward LSTM =================
    lp = ctx.enter_context(tc.tile_pool(name="lstm", bufs=1))
    Hf = lp.tile([128, 2 * ROWS], F16, name="Hf")
    Cf = lp.tile([128, 2 * ROWS], F16, name="Cf")
    nc.vector.memset(Hf[:], 0.0)
    nc.vector.memset(Cf[:], 0.0)
    gi = lp.tile([128, 2 * ROWS], F16, name="gi")
    gf = lp.tile([128, 2 * ROWS], F16, name="gf")
    go = lp.tile([128, 2 * ROWS], F16, name="go")
    tg = lp.tile([128, 2 * ROWS], F16, name="tg")
    tcl = lp.tile([128, 2 * ROWS], F16, name="tcl")
    tmp = lp.tile([128, 2 * ROWS], F16, name="tmp")
    gate_dst = [gi, gi, gf, gf, go, go, tg, tg]

    with tc.tile_pool(name="lps", bufs=3, space="PSUM") as ps_l:
        for s in range(WIN):
            k0, par = s // 2, s % 2
            for mt in range(8):
                ps = ps_l.tile([128, 1024], F32, tag="lps", name="lps")
                uv = Umt[mt][:].rearrange("p (b k two) -> p b k two",
                                          b=BL, two=2)
                for hh in range(2):
                    pslice = ps[:, hh * 512:hh * 512 + HROWS]
                    b0 = hh * (BL // 2)
                    nc.tensor.matmul(
                        pslice, ident[:],
                        uv[:, b0:b0 + BL // 2, k0:k0 + NW, par],
                        start=True, stop=False)
                    for kt in range(2):
                        nc.tensor.matmul(
                            pslice, lhf[kt][:, mt * 128:(mt + 1) * 128],
                            Hf[:, kt * ROWS + hh * HROWS:
                               kt * ROWS + (hh + 1) * HROWS],
                            start=False, stop=(kt == 1))
                dst = gate_dst[mt][:, (mt % 2) * ROWS:(mt % 2 + 1) * ROWS]
                dstv = dst.rearrange("p (h r) -> p h r", h=2)
                psv = ps[:].rearrange("p (h x) -> p h x", h=2)[:, :, 0:HROWS]
                fn = AF.Sigmoid if mt < 6 else AF.Tanh
                nc.scalar.activation(dstv, psv, fn,
                                     bias=bgf[:, mt:mt + 1], scale=1.0)
            nc.vector.tensor_tensor(tmp[:], gi[:], tg[:], ALU.mult)
            nc.vector.tensor_tensor(Cf[:], gf[:], Cf[:], ALU.mult)
            nc.vector.tensor_tensor(Cf[:], Cf[:], tmp[:], ALU.add)
            nc.scalar.activation(tcl[:], Cf[:], AF.Tanh)
            nc.vector.tensor_tensor(Hf[:], go[:], tcl[:], ALU.mult)

        # ===== Phase 4: backward LSTM single step (only hb[:,0] used) =====
        Hb = lp.tile([128, 2 * ROWS], F16, name="Hb")
        kb = (WIN - 2) // 2
        for mt in [0, 1, 4, 5, 6, 7]:          # forget gate irrelevant (c0=0)
            ps = ps_l.tile([128, 1024], F32, tag="lps", name="lpsb")
            for hh in range(2):
                pslice = ps[:, hh * 512:hh * 512 + HROWS]
                b0 = hh * (BL // 2)
                for kt in range(2):
                    fv = Fts[kt][:].rearrange("p (b k two) -> p b k two",
                                              b=BL, two=2)
                    nc.tensor.matmul(
                        pslice, lxb[kt][:, mt * 128:(mt + 1) * 128],
                        fv[:, b0:b0 + BL // 2, kb:kb + NW, 1],
                        start=(kt == 0), stop=(kt == 1))
            dst = gate_dst[mt][:, (mt % 2) * ROWS:(mt % 2 + 1) * ROWS]
            dstv = dst.rearrange("p (h r) -> p h r", h=2)
            psv = ps[:].rearrange("p (h x) -> p h x", h=2)[:, :, 0:HROWS]
            fn = AF.Sigmoid if mt < 6 else AF.Tanh
            nc.scalar.activation(dstv, psv, fn,
                                 bias=bgb[:, mt:mt + 1], scale=1.0)
        nc.vector.tensor_tensor(tmp[:], gi[:], tg[:], ALU.mult)
        nc.scalar.activation(tcl[:], tmp[:], AF.Tanh)
        nc.vector.tensor_tensor(Hb[:], go[:], tcl[:], ALU.mult)

        # ===== Phase 5: FC head =====
        ps = ps_l.tile([128, 1024], F32, tag="lps", name="lpsf")
        rhs4 = [Hf[:, 0:ROWS], Hf[:, ROWS:2 * ROWS],
                Hb[:, 0:ROWS], Hb[:, ROWS:2 * ROWS]]
        for hh in range(2):
            for qt in range(4):
                nc.tensor.matmul(ps[:, hh * 512:hh * 512 + HROWS],
                                 wfct[qt][:],
                                 rhs4[qt].rearrange("p (h r) -> p h r",
                                                    h=2)[:, hh, :],
                                 start=(qt == 0), stop=(qt == 3))
        ob = lp.tile([EMB, ROWS], F32, name="ob")
        obv = ob[:].rearrange("p (h r) -> p h r", h=2)
        psv = ps[:].rearrange("p (h x) -> p h x", h=2)[:, :, 0:HROWS]
        nc.scalar.activation(obv, psv, AF.Identity,
                             bias=bfc[:, 0:1], scale=1.0)
        nc.sync.dma_start(io["out_d"][:], ob[:])

    ctx.close()


def _build_program():
    nc = bacc.Bacc("TRN2", target_bir_lowering=False, debug=False,
                   num_devices=NCORES)

    def din(name, shape, dt=F16):
        return nc.dram_tensor(name, shape, dt, kind="ExternalInput").ap()

    io = dict(
        x0B=din("x0B", [NCH, 128, GBLK * 128]),
        mixM=din("mixM", [128, 128]),
        w1=din("w1", [FIN, 128]), w2=din("w2", [64, 128]),
        w3=din("w3", [128, 256]), w4=din("w4", [256, 256]),
        b1=din("b1", [128, 1], F32), b2=din("b2", [128, 1], F32),
        b3=din("b3", [128, 2], F32), b4=din("b4", [128, 2], F32),
        lxf=din("lxf", [256, 1024]), lhf=din("lhf", [256, 1024]),
        lxb=din("lxb", [256, 1024]),
        bgf=din("bgf", [128, 8], F32), bgb=din("bgb", [128, 8], F32),
        wfc=din("wfc", [512, 128]), bfc=din("bfc", [128, 1], F32),
        ident=din("ident", [128, 128]),
        out_d=nc.dram_tensor("out", [EMB, ROWS], F32,
                             kind="ExternalOutput").ap(),
    )
    with tile.TileContext(nc) as tc:
        _kernel_body(tc, io)
    nc.compile()
    return nc


def _host_prep(inputs):
    f16 = np.float16
    data = np.asarray(inputs["data"], np.float32)
    ei = np.asarray(inputs["edge_index"]).astype(np.int64)

    src = np.concatenate([ei[0], np.arange(N)])
    dst = np.concatenate([ei[1], np.arange(N)])
    deg = np.zeros(N, np.float32)
    np.add.at(deg, dst, 1.0)
    dinv = np.where(deg > 0, deg ** -0.5, 0.0).astype(np.float32)
    Ahat = np.zeros((N, N), np.float32)
    np.add.at(Ahat, (dst, src), dinv[src] * dinv[dst])
    mixM = np.zeros((128, 128), np.float32)
    mixM[0:N * G5, 0:N * G5] = np.kron(Ahat.T, np.eye(G5, dtype=np.float32))
    mixM = mixM.astype(f16)

    # x0B: [core][chunk b][blk = n*5+g5 (120:128 zero)][gb*128 + c],
    # t = 5*gb+g5, channels 6:128 zero
    d = data.reshape(NCORES, BL, T, N, FIN)
    x0B = np.zeros((NCORES, BL, 128, GBLK, 128), np.float32)
    dpad = np.zeros((NCORES, BL, TP, N, FIN), np.float32)
    dpad[:, :, :T] = d
    dv = dpad.reshape(NCORES, BL, GBLK, G5, N, FIN)
    # [core, b, n, g5, gb, c]
    dv = dv.transpose(0, 1, 4, 3, 2, 5).reshape(NCORES, BL, N * G5, GBLK, FIN)
    x0B[:, :, 0:N * G5, :, 0:FIN] = dv
    x0B = np.ascontiguousarray(
        x0B.reshape(NCORES, BL, 128, GBLK * 128)).astype(f16)

    perm = np.concatenate([np.arange(0, H), np.arange(H, 2 * H),
                           np.arange(3 * H, 4 * H), np.arange(2 * H, 3 * H)])

    def prep_dir(wih, whh, bih, bhh):
        wihp = np.asarray(wih, np.float32)[perm] / N
        whhp = np.asarray(whh, np.float32)[perm]
        bg = (np.asarray(bih, np.float32) + np.asarray(bhh, np.float32))[perm]
        return (np.ascontiguousarray(wihp.T).astype(f16),
                np.ascontiguousarray(whhp.T).astype(f16),
                np.ascontiguousarray(bg.reshape(8, 128).T).astype(np.float32))

    lxf, lhf, bgf = prep_dir(inputs["lstm_Wih_f"], inputs["lstm_Whh_f"],
                             inputs["lstm_bih_f"], inputs["lstm_bhh_f"])
    lxb, _lhb, bgb = prep_dir(inputs["lstm_Wih_b"], inputs["lstm_Whh_b"],
                              inputs["lstm_bih_b"], inputs["lstm_bhh_b"])

    com = {
        "mixM": mixM,
        "w1": np.pad(np.asarray(inputs["W1"], np.float32),
                     ((0, 0), (0, 64))).astype(f16),
        "w2": np.asarray(inputs["W2"], np.float32).astype(f16),
        "w3": np.asarray(inputs["W3"], np.float32).astype(f16),
        "w4": np.asarray(inputs["W4"], np.float32).astype(f16),
        "b1": np.pad(np.asarray(inputs["b1"], np.float32),
                     (0, 64)).reshape(128, 1),
        "b2": np.asarray(inputs["b2"], np.float32).reshape(128, 1),
        "b3": np.ascontiguousarray(
            np.asarray(inputs["b3"], np.float32).reshape(2, 128).T),
        "b4": np.ascontiguousarray(
            np.asarray(inputs["b4"], np.float32).reshape(2, 128).T),
        "lxf": lxf, "lhf": lhf, "lxb": lxb, "bgf": bgf, "bgb": bgb,
        "wfc": np.asarray(inputs["Wfc"], np.float32).astype(f16),
        "bfc": np.asarray(inputs["bfc"], np.float32).reshape(128, 1),
        "ident": np.eye(128, dtype=f16),
    }
    return [dict(com, x0B=x0B[c]) for c in range(NCORES)]


TRACE = False          # set by test harness to capture an NTFF profile


def kernel(**inputs) -> np.ndarray:
    if "nc" not in _CACHE:
        _CACHE["nc"] = _build_program()
    nc = _CACHE["nc"]
    in_maps = _host_prep(inputs)
    res = bass_utils.run_bass_kernel_spmd(nc, in_maps,
                                          core_ids=list(range(NCORES)),
                                          trace=TRACE)
    _CACHE["last_res"] = res
    outs = []
    for c in range(NCORES):
        o = res.results[c]["out"]                       # [128, 904]
        outs.append(o.reshape(EMB, BL, NW).transpose(1, 2, 0))
    return np.concatenate(outs, 0).astype(np.float32)   # [64, 113, 128]


if __name__ == "__main__":
    import reference
    ins = {k: np.asarray(v) for k, v in reference.setup_inputs().items()}
    out = kernel(**ins)
    print("kernel out", out.shape, out.dtype, float(np.abs(out).max()))



# revision 17
# speedup vs baseline: 1.7850x; 1.7850x over previous
"""Trainium2 Bass kernel for DeepConvGraphEncoderDownstream.

Model (per reference):
  4-layer GCN (shared dense 24x24 graph operator) applied per (batch, timestep)
  frame -> node-mean -> per sliding window (W=32, stride 2, 113 windows):
  BiLSTM(H=256) -> concat(h_fwd[-1], h_bwd[0]) @ Wfc + bfc.

Key algebraic restructurings:
  * gcn_norm folded into one dense Ahat[24,24] on host.
  * GCN runs ONCE over all 256 timesteps (the reference recomputes it ~14x
    across overlapping windows).
  * backward LSTM: only hb[:, 0] is used => exactly ONE step, no recurrence.
  * forward LSTM: all 113 windows batched into one 904-row recurrence per
    core; input transforms U precomputed per-chunk during the GCN phase.

Sharding: data-parallel over batch, 8 batches/core on 8 cores; output
slices are independent (no collectives).

GCN layout scheme (NO DMA transposes — the v1 kernel spent ~570us/core in
serialized XBAR DMA_TRANSPOSE ops):
  A-layout [c_part, free=(gb:52, blk:128)], blk = n*5+g5, t = 5*gb+g5.
  B-layout [blk_part, free=(gb:52, c)].
  Per GCN layer, one matmul runs "data-as-stationary" (lhsT = activation
  tile block, rhs = small operator) which flips layout A<->B as a side
  effect of out = lhsT.T @ rhs; the other matmul runs classic (operator
  stationary, activations moving).  Layer pairing:
    L1: transform-ds (A->B) + mix-classic  (B->B)   relu/bias via mix row
    L2: mix-ds      (B->A) + transform-cl (A->A)   relu+bias at eviction
    L3: transform-ds (A->B) + mix-classic  (B->B)   relu/bias via mix row
    L4: mix-ds      (B->A) + transform-cl (A->A)   relu+bias at eviction
  B-side bias: stationary mix matrix has an extra all-ones row 120 that
  multiplies a bias pattern pre-written into partition 120 of the B tile.
  Node-mean after L4: in-place DVE tree-sum over the n-stride-5 free dim,
  last add writes directly into the F tile.
PSUM evictions are spread across Vector/Scalar(Act)/GpSimd(Pool) engines.
"""

import os
import sys
import numpy as np

try:
    import concourse.bass as bass
except ImportError:
    sys.path.insert(0, "/opt/trn_rl_repo")
    import concourse.bass as bass

import concourse.bacc as bacc
import concourse.tile as tile
from concourse import mybir
from concourse import bass_utils

F16 = mybir.dt.float16
F32 = mybir.dt.float32
AF = mybir.ActivationFunctionType
ALU = mybir.AluOpType

B, T, N, FIN = 64, 256, 24, 6
H, EMB = 256, 128
WIN = 32
NW = (T - WIN) // 2 + 1               # 113
NCORES = 8
BL = B // NCORES                      # 8
G5 = 5
GBLK = 52                             # 52*5 = 260 t-slots
TP = GBLK * G5                        # 260 padded timesteps
NCH = BL
ROWS = BL * NW                        # 904
HROWS = ROWS // 2                     # 452
CH_FREE = GBLK * 128                  # 6656 A-layout free per chunk
FTOT = BL * TP                        # 2080 F columns
NB = N * G5                           # 120 valid blk rows

_CACHE = {}


def _kernel_body(tc, io):
    nc = tc.nc
    from contextlib import ExitStack
    ctx = ExitStack()

    cons = ctx.enter_context(tc.tile_pool(name="cons", bufs=1))
    fpool = ctx.enter_context(tc.tile_pool(name="fpool", bufs=1))

    def load_const(name, shape, dt=F16):
        t = cons.tile(shape, dt, name=name)
        nc.sync.dma_start(t[:], io[name][:])
        return t

    mixM = load_const("mixM", [NB, 128])          # plain kron(Ahat^T,I5)
    mixMb1 = load_const("mixMb1", [NB + 1, 128])  # + bias row 120
    mixMb3 = load_const("mixMb3", [NB + 1, 128])
    w1 = load_const("w1", [FIN, 64])
    w2 = load_const("w2", [64, 128])
    w3 = load_const("w3", [128, 256])
    b2 = load_const("b2", [128, 1], F32)
    b4 = load_const("b4", [128, 2], F32)
    w4k = []
    for kt in range(2):
        t = cons.tile([128, 256], F16, name=f"w4k{kt}")
        nc.sync.dma_start(t[:], io["w4"][kt * 128:(kt + 1) * 128, :])
        w4k.append(t)

    def load_ktiles(pool, name):
        ts = []
        for kt in range(2):
            t = pool.tile([128, 1024], F16, name=f"{name}{kt}")
            nc.sync.dma_start(t[:], io[name][kt * 128:(kt + 1) * 128, :])
            ts.append(t)
        return ts

    lxf = load_ktiles(cons, "lxf")

    F0 = fpool.tile([128, FTOT], F16, name="F0")
    F1 = fpool.tile([128, FTOT], F16, name="F1")
    Fts = [F0, F1]
    Umt = [fpool.tile([128, FTOT], F16, name=f"U{mt}") for mt in range(8)]

    # ================= Phase 1: GCN + per-chunk U =================
    with tc.tile_pool(name="gx0", bufs=1) as gx0, \
         tc.tile_pool(name="gwork", bufs=1) as gw, \
         tc.tile_pool(name="psA", bufs=3, space="PSUM") as psA, \
         tc.tile_pool(name="psB", bufs=3, space="PSUM") as psB, \
         tc.tile_pool(name="psU", bufs=2, space="PSUM") as psU:

        # persistent per-stage work tiles (reused across chunks)
        B1 = gw.tile([NB + 1, GBLK * 64], F16, name="B1")
        B1p = gw.tile([NB, GBLK * 64], F16, name="B1p")
        A2 = gw.tile([64, CH_FREE], F16, name="A2")
        A2p = gw.tile([128, CH_FREE], F16, name="A2p")
        # B3 is 128-part so the t-cl4 outputs X4 can alias its storage
        # (B3's last read, mix-cl3, precedes the first X4 write in PE order)
        B3 = gw.tile([128, GBLK * 256], F16, name="B3")
        B3p = gw.tile([NB, GBLK * 256], F16, name="B3p")
        A4 = [gw.tile([128, CH_FREE], F16, name=f"A4_{h}") for h in range(2)]
        X4 = [B3[:, m * CH_FREE:(m + 1) * CH_FREE] for m in range(2)]

        # bias pattern into row 120 of B1 (once; B1 rows 0:120 only are
        # rewritten per chunk).  B3's row 120 is re-loaded per chunk since
        # the aliased X4 writes clobber it.
        nc.sync.dma_start(B1[NB:NB + 1, :], io["b1row"][:])

        def emit_U(k):
            # U_k = F_k @ (Wih_f/24)^T — emitted one chunk late so the
            # tree-sum feeding F_k has drained off the vector engine
            for mt in range(8):
                ps = psU.tile([128, 512], F32, tag="psU", name="psu")
                for kt in range(2):
                    nc.tensor.matmul(ps[:, 0:TP],
                                     lxf[kt][:, mt * 128:(mt + 1) * 128],
                                     Fts[kt][:, k * TP:(k + 1) * TP],
                                     start=(kt == 0), stop=(kt == 1))
                dst = Umt[mt][:, k * TP:(k + 1) * TP]
                if mt % 2:
                    nc.scalar.copy(dst, ps[:, 0:TP])
                else:
                    nc.vector.tensor_copy(dst, ps[:, 0:TP])

        for k in range(NCH):
            x0 = gx0.tile([FIN, CH_FREE], F16, tag="x0", name="x0")
            nc.sync.dma_start(x0[:], io["x0A"][k])
            nc.sync.dma_start(B3[NB:NB + 1, :], io["b3row"][:])

            # --- L1 transform-ds: per gb, out_B[blk, c'64] = x0_gb.T @ W1
            for bk in range(7):             # banks of 8 gb (last 4)
                ng = min(8, 52 - bk * 8)
                ps = psA.tile([128, 512], F32, tag="psA", name="ps1")
                for g in range(ng):
                    gb = bk * 8 + g
                    nc.tensor.matmul(ps[:, g * 64:(g + 1) * 64],
                                     x0[:, gb * 128:(gb + 1) * 128],
                                     w1[:],
                                     start=True, stop=True)
                nc.vector.tensor_copy(B1[0:NB, bk * 512:bk * 512 + ng * 64],
                                      ps[0:NB, 0:ng * 64])

            # --- L1 mix-classic (+bias row +relu): B1 -> B1p
            for fc in range(7):
                f0 = fc * 512
                fw = min(512, GBLK * 64 - f0)
                ps = psB.tile([128, 512], F32, tag="psB", name="ps1m")
                nc.tensor.matmul(ps[:, 0:fw], mixMb1[:],
                                 B1[:, f0:f0 + fw], start=True, stop=True)
                nc.scalar.activation(B1p[:, f0:f0 + fw], ps[0:NB, 0:fw],
                                     AF.Relu)

            # --- L2 mix-ds: per gb, out_A[c64, blk'] = B1p_gb.T @ mixM
            for bk in range(13):            # banks of 4 gb
                ps = psA.tile([128, 512], F32, tag="psA", name="ps2")
                for g in range(4):
                    gb = bk * 4 + g
                    nc.tensor.matmul(ps[0:64, g * 128:(g + 1) * 128],
                                     B1p[:, gb * 64:(gb + 1) * 64],
                                     mixM[:],
                                     start=True, stop=True)
                nc.vector.tensor_copy(A2[:, bk * 512:(bk + 1) * 512],
                                      ps[0:64, :])

            # --- L2 transform-cl (+bias+relu): A2 -> A2p
            for fc in range(13):
                f0 = fc * 512
                ps = psB.tile([128, 512], F32, tag="psB", name="ps2t")
                nc.tensor.matmul(ps[:], w2[:], A2[:, f0:f0 + 512],
                                 start=True, stop=True)
                nc.scalar.activation(A2p[:, f0:f0 + 512], ps[:], AF.Relu,
                                     bias=b2[:, 0:1], scale=1.0)

            if k > 0:
                emit_U(k - 1)

            # --- L3 transform-ds: per gb, out_B[blk, c'256] = A2p_gb.T @ W3
            for bk in range(26):            # banks of 2 gb
                ps = psA.tile([128, 512], F32, tag="psA", name="ps3")
                for g in range(2):
                    gb = bk * 2 + g
                    nc.tensor.matmul(ps[:, g * 256:(g + 1) * 256],
                                     A2p[:, gb * 128:(gb + 1) * 128],
                                     w3[:],
                                     start=True, stop=True)
                nc.vector.tensor_copy(B3[0:NB, bk * 512:(bk + 1) * 512],
                                      ps[0:NB, :])

            # --- L3 mix-classic (+bias row +relu): B3 -> B3p
            for fc in range(26):
                f0 = fc * 512
                ps = psB.tile([128, 512], F32, tag="psB", name="ps3m")
                nc.tensor.matmul(ps[:], mixMb3[:], B3[0:NB + 1, f0:f0 + 512],
                                 start=True, stop=True)
                nc.scalar.activation(B3p[:, f0:f0 + 512], ps[0:NB, :],
                                     AF.Relu)

            # --- L4 mix-ds: per (gb, c-half), out_A[c128, blk'] = B3p_gb_h.T @ mixM
            for h in range(2):
                for bk in range(13):
                    ps = psA.tile([128, 512], F32, tag="psA", name="ps4")
                    for g in range(4):
                        gb = bk * 4 + g
                        c0 = gb * 256 + h * 128
                        nc.tensor.matmul(ps[:, g * 128:(g + 1) * 128],
                                         B3p[:, c0:c0 + 128],
                                         mixM[:],
                                         start=True, stop=True)
                    dst = A4[h][:, bk * 512:(bk + 1) * 512]
                    if bk % 2 == 0:
                        nc.scalar.copy(dst, ps[:])
                    else:
                        nc.vector.tensor_copy(dst, ps[:])

            # --- L4 transform-cl (+bias+relu): A4 -> X4 (2 m-tiles)
            for mt in range(2):
                for fc in range(13):
                    f0 = fc * 512
                    ps = psB.tile([128, 512], F32, tag="psB", name="ps4t")
                    for kt in range(2):
                        nc.tensor.matmul(ps[:],
                                         w4k[kt][:, mt * 128:(mt + 1) * 128],
                                         A4[kt][:, f0:f0 + 512],
                                         start=(kt == 0), stop=(kt == 1))
                    nc.scalar.activation(X4[mt][:, f0:f0 + 512], ps[:],
                                         AF.Relu, bias=b4[:, mt:mt + 1],
                                         scale=1.0)

            # --- node-sum tree (24 nodes, stride 5 in blk) -> Fts chunk k
            for mt in range(2):
                xv = X4[mt][:].rearrange("p (gb blk) -> p gb blk", blk=128)
                nc.vector.tensor_tensor(xv[:, :, 0:60], xv[:, :, 0:60],
                                        xv[:, :, 60:120], ALU.add)
                nc.vector.tensor_tensor(xv[:, :, 0:30], xv[:, :, 0:30],
                                        xv[:, :, 30:60], ALU.add)
                nc.vector.tensor_tensor(xv[:, :, 0:15], xv[:, :, 0:15],
                                        xv[:, :, 15:30], ALU.add)
                nc.vector.tensor_tensor(xv[:, :, 0:5], xv[:, :, 0:5],
                                        xv[:, :, 5:10], ALU.add)
                dstv = Fts[mt][:, k * TP:(k + 1) * TP].rearrange(
                    "p (gb g5) -> p gb g5", g5=G5)
                nc.vector.tensor_tensor(dstv, xv[:, :, 0:5],
                                        xv[:, :, 10:15], ALU.add)

        emit_U(NCH - 1)

    # ===== LSTM-only constants (loaded after the GCN pools free SBUF) =====
    cons2 = ctx.enter_context(tc.tile_pool(name="cons2", bufs=1))
    lhf = load_ktiles(cons2, "lhf")
    lxb = load_ktiles(cons2, "lxb")

    def load_const2(name, shape, dt=F16):
        t = cons2.tile(shape, dt, name=name)
        nc.sync.dma_start(t[:], io[name][:])
        return t

    bgf = load_const2("bgf", [128, 8], F32)
    bgb = load_const2("bgb", [128, 8], F32)
    ident = load_const2("ident", [128, 128])
    wfct = []
    for qt in range(4):
        t = cons2.tile([128, 128], F16, name=f"wfct{qt}")
        nc.sync.dma_start(t[:], io["wfc"][qt * 128:(qt + 1) * 128, :])
        wfct.append(t)
    bfc = load_const2("bfc", [128, 1], F32)

    # ================= Phase 3: forward LSTM =================
    lp = ctx.enter_context(tc.tile_pool(name="lstm", bufs=1))
    Hf = lp.tile([128, 2 * ROWS], F16, name="Hf")
    Cf = lp.tile([128, 2 * ROWS], F16, name="Cf")
    nc.vector.memset(Hf[:], 0.0)
    nc.vector.memset(Cf[:], 0.0)
    gi = lp.tile([128, 2 * ROWS], F16, name="gi")
    gf = lp.tile([128, 2 * ROWS], F16, name="gf")
    go = lp.tile([128, 2 * ROWS], F16, name="go")
    tg = lp.tile([128, 2 * ROWS], F16, name="tg")
    tcl = lp.tile([128, 2 * ROWS], F16, name="tcl")
    tmp = lp.tile([128, 2 * ROWS], F16, name="tmp")
    gate_dst = [gi, gi, gf, gf, go, go, tg, tg]

    with tc.tile_pool(name="lps", bufs=3, space="PSUM") as ps_l:
        for s in range(WIN):
            k0, par = s // 2, s % 2
            for mt in range(8):
                ps = ps_l.tile([128, 1024], F32, tag="lps", name="lps")
                uv = Umt[mt][:].rearrange("p (b k two) -> p b k two",
                                          b=BL, two=2)
                for hh in range(2):
                    pslice = ps[:, hh * 512:hh * 512 + HROWS]
                    b0 = hh * (BL // 2)
                    nc.tensor.matmul(
                        pslice, ident[:],
                        uv[:, b0:b0 + BL // 2, k0:k0 + NW, par],
                        start=True, stop=False)
                    for kt in range(2):
                        nc.tensor.matmul(
                            pslice, lhf[kt][:, mt * 128:(mt + 1) * 128],
                            Hf[:, kt * ROWS + hh * HROWS:
                               kt * ROWS + (hh + 1) * HROWS],
                            start=False, stop=(kt == 1))
                dst = gate_dst[mt][:, (mt % 2) * ROWS:(mt % 2 + 1) * ROWS]
                dstv = dst.rearrange("p (h r) -> p h r", h=2)
                psv = ps[:].rearrange("p (h x) -> p h x", h=2)[:, :, 0:HROWS]
                fn = AF.Sigmoid if mt < 6 else AF.Tanh
                nc.scalar.activation(dstv, psv, fn,
                                     bias=bgf[:, mt:mt + 1], scale=1.0)
            nc.vector.tensor_tensor(tmp[:], gi[:], tg[:], ALU.mult)
            nc.vector.tensor_tensor(Cf[:], gf[:], Cf[:], ALU.mult)
            nc.vector.tensor_tensor(Cf[:], Cf[:], tmp[:], ALU.add)
            nc.scalar.activation(tcl[:], Cf[:], AF.Tanh)
            nc.vector.tensor_tensor(Hf[:], go[:], tcl[:], ALU.mult)

        # ===== Phase 4: backward LSTM single step (only hb[:,0] used) =====
        Hb = lp.tile([128, 2 * ROWS], F16, name="Hb")
        kb = (WIN - 2) // 2
        for mt in [0, 1, 4, 5, 6, 7]:          # forget gate irrelevant (c0=0)
            ps = ps_l.tile([128, 1024], F32, tag="lps", name="lpsb")
            for hh in range(2):
                pslice = ps[:, hh * 512:hh * 512 + HROWS]
                b0 = hh * (BL // 2)
                for kt in range(2):
                    fv = Fts[kt][:].rearrange("p (b k two) -> p b k two",
                                              b=BL, two=2)
                    nc.tensor.matmul(
                        pslice, lxb[kt][:, mt * 128:(mt + 1) * 128],
                        fv[:, b0:b0 + BL // 2, kb:kb + NW, 1],
                        start=(kt == 0), stop=(kt == 1))
            dst = gate_dst[mt][:, (mt % 2) * ROWS:(mt % 2 + 1) * ROWS]
            dstv = dst.rearrange("p (h r) -> p h r", h=2)
            psv = ps[:].rearrange("p (h x) -> p h x", h=2)[:, :, 0:HROWS]
            fn = AF.Sigmoid if mt < 6 else AF.Tanh
            nc.scalar.activation(dstv, psv, fn,
                                 bias=bgb[:, mt:mt + 1], scale=1.0)
        nc.vector.tensor_tensor(tmp[:], gi[:], tg[:], ALU.mult)
        nc.scalar.activation(tcl[:], tmp[:], AF.Tanh)
        nc.vector.tensor_tensor(Hb[:], go[:], tcl[:], ALU.mult)

        # ===== Phase 5: FC head =====
        ps = ps_l.tile([128, 1024], F32, tag="lps", name="lpsf")
        rhs4 = [Hf[:, 0:ROWS], Hf[:, ROWS:2 * ROWS],
                Hb[:, 0:ROWS], Hb[:, ROWS:2 * ROWS]]
        for hh in range(2):
            for qt in range(4):
                nc.tensor.matmul(ps[:, hh * 512:hh * 512 + HROWS],
                                 wfct[qt][:],
                                 rhs4[qt].rearrange("p (h r) -> p h r",
                                                    h=2)[:, hh, :],
                                 start=(qt == 0), stop=(qt == 3))
        ob = lp.tile([EMB, ROWS], F32, name="ob")
        obv = ob[:].rearrange("p (h r) -> p h r", h=2)
        psv = ps[:].rearrange("p (h x) -> p h x", h=2)[:, :, 0:HROWS]
        nc.scalar.activation(obv, psv, AF.Identity,
                             bias=bfc[:, 0:1], scale=1.0)
        nc.sync.dma_start(io["out_d"][:], ob[:])

    ctx.close()


def _build_program():
    nc = bacc.Bacc("TRN2", target_bir_lowering=False, debug=False,
                   num_devices=NCORES)

    def din(name, shape, dt=F16):
        return nc.dram_tensor(name, shape, dt, kind="ExternalInput").ap()

    io = dict(
        x0A=din("x0A", [NCH, FIN, CH_FREE]),
        mixM=din("mixM", [NB, 128]),
        mixMb1=din("mixMb1", [NB + 1, 128]),
        mixMb3=din("mixMb3", [NB + 1, 128]),
        b1row=din("b1row", [1, GBLK * 64]),
        b3row=din("b3row", [1, GBLK * 256]),
        w1=din("w1", [FIN, 64]), w2=din("w2", [64, 128]),
        w3=din("w3", [128, 256]), w4=din("w4", [256, 256]),
        b2=din("b2", [128, 1], F32), b4=din("b4", [128, 2], F32),
        lxf=din("lxf", [256, 1024]), lhf=din("lhf", [256, 1024]),
        lxb=din("lxb", [256, 1024]),
        bgf=din("bgf", [128, 8], F32), bgb=din("bgb", [128, 8], F32),
        wfc=din("wfc", [512, 128]), bfc=din("bfc", [128, 1], F32),
        ident=din("ident", [128, 128]),
        out_d=nc.dram_tensor("out", [EMB, ROWS], F32,
                             kind="ExternalOutput").ap(),
    )
    with tile.TileContext(nc) as tc:
        _kernel_body(tc, io)
    nc.compile()
    return nc


def _host_prep(inputs):
    f16 = np.float16
    data = np.asarray(inputs["data"], np.float32)
    ei = np.asarray(inputs["edge_index"]).astype(np.int64)

    src = np.concatenate([ei[0], np.arange(N)])
    dst = np.concatenate([ei[1], np.arange(N)])
    deg = np.zeros(N, np.float32)
    np.add.at(deg, dst, 1.0)
    dinv = np.where(deg > 0, deg ** -0.5, 0.0).astype(np.float32)
    Ahat = np.zeros((N, N), np.float32)
    np.add.at(Ahat, (dst, src), dinv[src] * dinv[dst])
    mixM = np.kron(Ahat.T, np.eye(G5, dtype=np.float32)).astype(f16)  # [120,120]
    mixMp = np.zeros((NB, 128), f16)
    mixMp[:, 0:NB] = mixM

    def mixMb(bias_unused):
        m = np.zeros((NB + 1, 128), f16)
        m[0:NB, 0:NB] = mixM
        m[NB, 0:NB] = 1.0
        return m

    b1 = np.asarray(inputs["b1"], np.float32)
    b3 = np.asarray(inputs["b3"], np.float32)
    b1row = np.tile(b1[None, :], (GBLK, 1)).reshape(1, GBLK * 64).astype(f16)
    b3row = np.tile(b3[None, :], (GBLK, 1)).reshape(1, GBLK * 256).astype(f16)

    # x0A: [core][chunk b][c 6][gb*128 + blk], blk = n*5+g5, t = 5*gb+g5
    d = data.reshape(NCORES, BL, T, N, FIN)
    dpad = np.zeros((NCORES, BL, TP, N, FIN), np.float32)
    dpad[:, :, :T] = d
    dv = dpad.reshape(NCORES, BL, GBLK, G5, N, FIN)
    # -> [core, b, f, gb, n, g5]
    dv = dv.transpose(0, 1, 5, 2, 4, 3).reshape(NCORES, BL, FIN, GBLK, NB)
    x0A = np.zeros((NCORES, BL, FIN, GBLK, 128), np.float32)
    x0A[:, :, :, :, 0:NB] = dv
    x0A = np.ascontiguousarray(
        x0A.reshape(NCORES, BL, FIN, CH_FREE)).astype(f16)

    perm = np.concatenate([np.arange(0, H), np.arange(H, 2 * H),
                           np.arange(3 * H, 4 * H), np.arange(2 * H, 3 * H)])

    def prep_dir(wih, whh, bih, bhh):
        wihp = np.asarray(wih, np.float32)[perm] / N
        whhp = np.asarray(whh, np.float32)[perm]
        bg = (np.asarray(bih, np.float32) + np.asarray(bhh, np.float32))[perm]
        return (np.ascontiguousarray(wihp.T).astype(f16),
                np.ascontiguousarray(whhp.T).astype(f16),
                np.ascontiguousarray(bg.reshape(8, 128).T).astype(np.float32))

    lxf, lhf, bgf = prep_dir(inputs["lstm_Wih_f"], inputs["lstm_Whh_f"],
                             inputs["lstm_bih_f"], inputs["lstm_bhh_f"])
    lxb, _lhb, bgb = prep_dir(inputs["lstm_Wih_b"], inputs["lstm_Whh_b"],
                              inputs["lstm_bih_b"], inputs["lstm_bhh_b"])

    com = {
        "mixM": mixMp,
        "mixMb1": mixMb(None),
        "mixMb3": mixMb(None),
        "b1row": b1row,
        "b3row": b3row,
        "w1": np.asarray(inputs["W1"], np.float32).astype(f16),
        "w2": np.asarray(inputs["W2"], np.float32).astype(f16),
        "w3": np.asarray(inputs["W3"], np.float32).astype(f16),
        "w4": np.asarray(inputs["W4"], np.float32).astype(f16),
        "b2": np.asarray(inputs["b2"], np.float32).reshape(128, 1),
        "b4": np.ascontiguousarray(
            np.asarray(inputs["b4"], np.float32).reshape(2, 128).T),
        "lxf": lxf, "lhf": lhf, "lxb": lxb, "bgf": bgf, "bgb": bgb,
        "wfc": np.asarray(inputs["Wfc"], np.float32).astype(f16),
        "bfc": np.asarray(inputs["bfc"], np.float32).reshape(128, 1),
        "ident": np.eye(128, dtype=f16),
    }
    return [dict(com, x0A=x0A[c]) for c in range(NCORES)]


TRACE = False          # set by test harness to capture an NTFF profile


def kernel(**inputs) -> np.ndarray:
    if "nc" not in _CACHE:
        _CACHE["nc"] = _build_program()
    nc = _CACHE["nc"]
    in_maps = _host_prep(inputs)
    res = bass_utils.run_bass_kernel_spmd(nc, in_maps,
                                          core_ids=list(range(NCORES)),
                                          trace=TRACE)
    _CACHE["last_res"] = res
    outs = []
    for c in range(NCORES):
        o = res.results[c]["out"]                       # [128, 904]
        outs.append(o.reshape(EMB, BL, NW).transpose(1, 2, 0))
    return np.concatenate(outs, 0).astype(np.float32)   # [64, 113, 128]


if __name__ == "__main__":
    import reference
    ins = {k: np.asarray(v) for k, v in reference.setup_inputs().items()}
    out = kernel(**ins)
    print("kernel out", out.shape, out.dtype, float(np.abs(out).max()))


# revision 21
# speedup vs baseline: 2.1911x; 1.2275x over previous
"""Trainium2 Bass kernel for DeepConvGraphEncoderDownstream.

Model (per reference):
  4-layer GCN (shared dense 24x24 graph operator) applied per (batch, timestep)
  frame -> node-mean -> per sliding window (W=32, stride 2, 113 windows):
  BiLSTM(H=256) -> concat(h_fwd[-1], h_bwd[0]) @ Wfc + bfc.

Key algebraic restructurings:
  * gcn_norm folded into one dense Ahat[24,24] on host.
  * GCN runs ONCE over all 256 timesteps (the reference recomputes it ~14x
    across overlapping windows).
  * backward LSTM: only hb[:, 0] is used => exactly ONE step, no recurrence.
  * forward LSTM: all 113 windows batched into one 904-row recurrence per
    core; input transforms U precomputed per-chunk during the GCN phase.

Sharding: data-parallel over batch, 8 batches/core on 8 cores; output
slices are independent (no collectives).

GCN layout scheme (NO DMA transposes — the v1 kernel spent ~570us/core in
serialized XBAR DMA_TRANSPOSE ops):
  A-layout [c_part, free=(gb:52, blk:128)], blk = n*5+g5, t = 5*gb+g5.
  B-layout [blk_part, free=(gb:52, c)].
  Per GCN layer, one matmul runs "data-as-stationary" (lhsT = activation
  tile block, rhs = small operator) which flips layout A<->B as a side
  effect of out = lhsT.T @ rhs; the other matmul runs classic (operator
  stationary, activations moving).  Layer pairing:
    L1: transform-ds (A->B) + mix-classic  (B->B)   relu/bias via mix row
    L2: mix-ds      (B->A) + transform-cl (A->A)   relu+bias at eviction
    L3: transform-ds (A->B) + mix-classic  (B->B)   relu/bias via mix row
    L4: mix-ds      (B->A) + transform-cl (A->A)   relu+bias at eviction
  B-side bias: stationary mix matrix has an extra all-ones row 120 that
  multiplies a bias pattern pre-written into partition 120 of the B tile.
  Node-mean after L4: in-place DVE tree-sum over the n-stride-5 free dim,
  last add writes directly into the F tile.
PSUM evictions are spread across Vector/Scalar(Act)/GpSimd(Pool) engines.
"""

import os
import sys
import numpy as np

try:
    import concourse.bass as bass
except ImportError:
    sys.path.insert(0, "/opt/trn_rl_repo")
    import concourse.bass as bass

import concourse.bacc as bacc
import concourse.tile as tile
from concourse import mybir
from concourse import bass_utils

F16 = mybir.dt.float16
F32 = mybir.dt.float32
AF = mybir.ActivationFunctionType
ALU = mybir.AluOpType

B, T, N, FIN = 64, 256, 24, 6
H, EMB = 256, 128
WIN = 32
NW = (T - WIN) // 2 + 1               # 113
NCORES = 8
BL = B // NCORES                      # 8
G5 = 5
GBLK = 52                             # 52*5 = 260 t-slots
TP = GBLK * G5                        # 260 padded timesteps
NCH = BL
ROWS = BL * NW                        # 904
HROWS = ROWS // 2                     # 452
CH_FREE = GBLK * 128                  # 6656 A-layout free per chunk
FTOT = BL * TP                        # 2080 F columns
NB = N * G5                           # 120 valid blk rows

_CACHE = {}


def _kernel_body(tc, io):
    nc = tc.nc
    from contextlib import ExitStack
    ctx = ExitStack()

    cons = ctx.enter_context(tc.tile_pool(name="cons", bufs=1))
    fpool = ctx.enter_context(tc.tile_pool(name="fpool", bufs=1))

    def load_const(name, shape, dt=F16):
        t = cons.tile(shape, dt, name=name)
        nc.sync.dma_start(t[:], io[name][:])
        return t

    mixM = load_const("mixM", [NB, 128])          # plain kron(Ahat^T,I5)
    mixMb1 = load_const("mixMb1", [NB + 1, 128])  # + bias row 120
    mixMb3 = load_const("mixMb3", [NB + 1, 128])
    w1 = load_const("w1", [FIN, 64])
    w2 = load_const("w2", [64, 128])
    w3 = load_const("w3", [128, 256])
    b2 = load_const("b2", [128, 1], F32)
    b4 = load_const("b4", [128, 2], F32)
    w4k = []
    for kt in range(2):
        t = cons.tile([128, 256], F16, name=f"w4k{kt}")
        nc.sync.dma_start(t[:], io["w4"][kt * 128:(kt + 1) * 128, :])
        w4k.append(t)

    def load_ktiles(pool, name):
        ts = []
        for kt in range(2):
            t = pool.tile([128, 1024], F16, name=f"{name}{kt}")
            nc.sync.dma_start(t[:], io[name][kt * 128:(kt + 1) * 128, :])
            ts.append(t)
        return ts

    lxf = load_ktiles(cons, "lxf")

    F0 = fpool.tile([128, FTOT], F16, name="F0")
    F1 = fpool.tile([128, FTOT], F16, name="F1")
    Fts = [F0, F1]
    Umt = [fpool.tile([128, FTOT], F16, name=f"U{mt}") for mt in range(8)]

    # ================= Phase 1: GCN + per-chunk U =================
    with tc.tile_pool(name="gx0", bufs=1) as gx0, \
         tc.tile_pool(name="gwork", bufs=1) as gw, \
         tc.tile_pool(name="psA", bufs=3, space="PSUM") as psA, \
         tc.tile_pool(name="psB", bufs=3, space="PSUM") as psB, \
         tc.tile_pool(name="psU", bufs=2, space="PSUM") as psU:

        # persistent per-stage work tiles (reused across chunks)
        B1 = gw.tile([NB + 1, GBLK * 64], F16, name="B1")
        B1p = gw.tile([NB, GBLK * 64], F16, name="B1p")
        A2 = gw.tile([64, CH_FREE], F16, name="A2")
        A2p = gw.tile([128, CH_FREE], F16, name="A2p")
        # B3 is 128-part so the t-cl4 outputs X4 can alias its storage
        # (B3's last read, mix-cl3, precedes the first X4 write in PE order)
        B3 = gw.tile([128, GBLK * 256], F16, name="B3")
        B3p = gw.tile([NB, GBLK * 256], F16, name="B3p")
        A4 = [gw.tile([128, CH_FREE], F16, name=f"A4_{h}") for h in range(2)]
        X4 = [B3[:, m * CH_FREE:(m + 1) * CH_FREE] for m in range(2)]

        # bias pattern into row 120 of B1 (once; B1 rows 0:120 only are
        # rewritten per chunk).  B3's row 120 is re-loaded per chunk since
        # the aliased X4 writes clobber it.
        nc.sync.dma_start(B1[NB:NB + 1, :], io["b1row"][:])

        def emit_U(k):
            # U_k = F_k @ (Wih_f/24)^T — emitted one chunk late so the
            # tree-sum feeding F_k has drained off the vector engine
            for mt in range(8):
                ps = psU.tile([128, 512], F32, tag="psU", name="psu")
                for kt in range(2):
                    nc.tensor.matmul(ps[:, 0:TP],
                                     lxf[kt][:, mt * 128:(mt + 1) * 128],
                                     Fts[kt][:, k * TP:(k + 1) * TP],
                                     start=(kt == 0), stop=(kt == 1))
                dst = Umt[mt][:, k * TP:(k + 1) * TP]
                if mt % 2:
                    nc.scalar.copy(dst, ps[:, 0:TP])
                else:
                    nc.vector.tensor_copy(dst, ps[:, 0:TP])

        for k in range(NCH):
            x0 = gx0.tile([FIN, CH_FREE], F16, tag="x0", name="x0")
            nc.sync.dma_start(x0[:], io["x0A"][k])
            nc.sync.dma_start(B3[NB:NB + 1, :], io["b3row"][:])

            # --- L1 transform-ds: per gb, out_B[blk, c'64] = x0_gb.T @ W1
            for bk in range(7):             # banks of 8 gb (last 4)
                ng = min(8, 52 - bk * 8)
                ps = psA.tile([128, 512], F32, tag="psA", name="ps1")
                for g in range(ng):
                    gb = bk * 8 + g
                    nc.tensor.matmul(ps[:, g * 64:(g + 1) * 64],
                                     x0[:, gb * 128:(gb + 1) * 128],
                                     w1[:],
                                     start=True, stop=True)
                nc.vector.tensor_copy(B1[0:NB, bk * 512:bk * 512 + ng * 64],
                                      ps[0:NB, 0:ng * 64])

            # --- L1 mix-classic (+bias row +relu): B1 -> B1p
            for fc in range(7):
                f0 = fc * 512
                fw = min(512, GBLK * 64 - f0)
                ps = psB.tile([128, 512], F32, tag="psB", name="ps1m")
                nc.tensor.matmul(ps[:, 0:fw], mixMb1[:],
                                 B1[:, f0:f0 + fw], start=True, stop=True)
                nc.scalar.activation(B1p[:, f0:f0 + fw], ps[0:NB, 0:fw],
                                     AF.Relu)

            # --- L2 mix-ds: per gb, out_A[c64, blk'] = B1p_gb.T @ mixM
            for bk in range(13):            # banks of 4 gb
                ps = psA.tile([128, 512], F32, tag="psA", name="ps2")
                for g in range(4):
                    gb = bk * 4 + g
                    nc.tensor.matmul(ps[0:64, g * 128:(g + 1) * 128],
                                     B1p[:, gb * 64:(gb + 1) * 64],
                                     mixM[:],
                                     start=True, stop=True)
                nc.vector.tensor_copy(A2[:, bk * 512:(bk + 1) * 512],
                                      ps[0:64, :])

            # --- L2 transform-cl (+bias+relu): A2 -> A2p
            for fc in range(13):
                f0 = fc * 512
                ps = psB.tile([128, 512], F32, tag="psB", name="ps2t")
                nc.tensor.matmul(ps[:], w2[:], A2[:, f0:f0 + 512],
                                 start=True, stop=True)
                nc.scalar.activation(A2p[:, f0:f0 + 512], ps[:], AF.Relu,
                                     bias=b2[:, 0:1], scale=1.0)

            if k > 0:
                emit_U(k - 1)

            # --- L3 transform-ds: per gb, out_B[blk, c'256] = A2p_gb.T @ W3
            for bk in range(26):            # banks of 2 gb
                ps = psA.tile([128, 512], F32, tag="psA", name="ps3")
                for g in range(2):
                    gb = bk * 2 + g
                    nc.tensor.matmul(ps[:, g * 256:(g + 1) * 256],
                                     A2p[:, gb * 128:(gb + 1) * 128],
                                     w3[:],
                                     start=True, stop=True)
                nc.vector.tensor_copy(B3[0:NB, bk * 512:(bk + 1) * 512],
                                      ps[0:NB, :])

            # --- L3 mix-classic (+bias row +relu): B3 -> B3p
            for fc in range(26):
                f0 = fc * 512
                ps = psB.tile([128, 512], F32, tag="psB", name="ps3m")
                nc.tensor.matmul(ps[:], mixMb3[:], B3[0:NB + 1, f0:f0 + 512],
                                 start=True, stop=True)
                nc.scalar.activation(B3p[:, f0:f0 + 512], ps[0:NB, :],
                                     AF.Relu)

            # --- L4 mix-ds: per (gb, c-half), out_A[c128, blk'] = B3p_gb_h.T @ mixM
            for h in range(2):
                for bk in range(13):
                    ps = psA.tile([128, 512], F32, tag="psA", name="ps4")
                    for g in range(4):
                        gb = bk * 4 + g
                        c0 = gb * 256 + h * 128
                        nc.tensor.matmul(ps[:, g * 128:(g + 1) * 128],
                                         B3p[:, c0:c0 + 128],
                                         mixM[:],
                                         start=True, stop=True)
                    dst = A4[h][:, bk * 512:(bk + 1) * 512]
                    if bk % 2 == 0:
                        nc.scalar.copy(dst, ps[:])
                    else:
                        nc.vector.tensor_copy(dst, ps[:])

            # --- L4 transform-cl (+bias+relu): A4 -> X4 (2 m-tiles)
            for mt in range(2):
                for fc in range(13):
                    f0 = fc * 512
                    ps = psB.tile([128, 512], F32, tag="psB", name="ps4t")
                    for kt in range(2):
                        nc.tensor.matmul(ps[:],
                                         w4k[kt][:, mt * 128:(mt + 1) * 128],
                                         A4[kt][:, f0:f0 + 512],
                                         start=(kt == 0), stop=(kt == 1))
                    nc.scalar.activation(X4[mt][:, f0:f0 + 512], ps[:],
                                         AF.Relu, bias=b4[:, mt:mt + 1],
                                         scale=1.0)

            # --- node-sum tree (24 nodes, stride 5 in blk) -> Fts chunk k
            for mt in range(2):
                xv = X4[mt][:].rearrange("p (gb blk) -> p gb blk", blk=128)
                nc.vector.tensor_tensor(xv[:, :, 0:60], xv[:, :, 0:60],
                                        xv[:, :, 60:120], ALU.add)
                nc.vector.tensor_tensor(xv[:, :, 0:30], xv[:, :, 0:30],
                                        xv[:, :, 30:60], ALU.add)
                nc.vector.tensor_tensor(xv[:, :, 0:15], xv[:, :, 0:15],
                                        xv[:, :, 15:30], ALU.add)
                nc.vector.tensor_tensor(xv[:, :, 0:5], xv[:, :, 0:5],
                                        xv[:, :, 5:10], ALU.add)
                dstv = Fts[mt][:, k * TP:(k + 1) * TP].rearrange(
                    "p (gb g5) -> p gb g5", g5=G5)
                nc.vector.tensor_tensor(dstv, xv[:, :, 0:5],
                                        xv[:, :, 10:15], ALU.add)

        emit_U(NCH - 1)

    # ===== LSTM-only constants (loaded after the GCN pools free SBUF) =====
    cons2 = ctx.enter_context(tc.tile_pool(name="cons2", bufs=1))
    lhf = load_ktiles(cons2, "lhf")
    lxb = load_ktiles(cons2, "lxb")

    def load_const2(name, shape, dt=F16):
        t = cons2.tile(shape, dt, name=name)
        nc.sync.dma_start(t[:], io[name][:])
        return t

    bgf = load_const2("bgf", [128, 8], F32)
    bgb = load_const2("bgb", [128, 8], F32)
    ident = load_const2("ident", [128, 128])
    wfct = []
    for qt in range(4):
        t = cons2.tile([128, 128], F16, name=f"wfct{qt}")
        nc.sync.dma_start(t[:], io["wfc"][qt * 128:(qt + 1) * 128, :])
        wfct.append(t)
    bfc = load_const2("bfc", [128, 1], F32)

    # ================= Phase 3: forward LSTM =================
    lp = ctx.enter_context(tc.tile_pool(name="lstm", bufs=1))
    Hf = lp.tile([128, 2 * ROWS], F16, name="Hf")
    Cf = lp.tile([128, 2 * ROWS], F16, name="Cf")
    nc.vector.memset(Hf[:], 0.0)
    nc.vector.memset(Cf[:], 0.0)
    gi = lp.tile([128, 2 * ROWS], F16, name="gi")
    gf = lp.tile([128, 2 * ROWS], F16, name="gf")
    go = lp.tile([128, 2 * ROWS], F16, name="go")
    tg = lp.tile([128, 2 * ROWS], F16, name="tg")
    tcl = lp.tile([128, 2 * ROWS], F16, name="tcl")
    tmp = lp.tile([128, 2 * ROWS], F16, name="tmp")
    gate_dst = [gi, gi, gf, gf, go, go, tg, tg]

    def hs(t, hh):
        # [128, (g:2, hh:2, 452)] -> the hh half across both 904-col groups
        return t[:].rearrange("p (g h r) -> p g h r", g=2,
                              r=HROWS)[:, :, hh, :]

    with tc.tile_pool(name="lps", bufs=4, space="PSUM") as ps_l, \
         tc.tile_pool(name="lpsb", bufs=2, space="PSUM") as ps_b:
        # two independent row-half recurrences, software-pipelined so the
        # activation/combine tail of one half hides under the other's matmuls
        for s in range(WIN):
            k0, par = s // 2, s % 2
            for hh in range(2):
                b0 = hh * (BL // 2)
                for mt in range(8):
                    ps = ps_l.tile([128, 512], F32, tag="lps", name="lps")
                    uv = Umt[mt][:].rearrange("p (b k two) -> p b k two",
                                              b=BL, two=2)
                    nc.tensor.matmul(
                        ps[:, 0:HROWS], ident[:],
                        uv[:, b0:b0 + BL // 2, k0:k0 + NW, par],
                        start=True, stop=False)
                    for kt in range(2):
                        nc.tensor.matmul(
                            ps[:, 0:HROWS],
                            lhf[kt][:, mt * 128:(mt + 1) * 128],
                            Hf[:, kt * ROWS + hh * HROWS:
                               kt * ROWS + (hh + 1) * HROWS],
                            start=False, stop=(kt == 1))
                    dst = gate_dst[mt][:, (mt % 2) * ROWS + hh * HROWS:
                                       (mt % 2) * ROWS + (hh + 1) * HROWS]
                    fn = AF.Sigmoid if mt < 6 else AF.Tanh
                    nc.scalar.activation(dst, ps[:, 0:HROWS], fn,
                                         bias=bgf[:, mt:mt + 1], scale=1.0)
                nc.vector.tensor_tensor(hs(tmp, hh), hs(gi, hh),
                                        hs(tg, hh), ALU.mult)
                nc.vector.tensor_tensor(hs(Cf, hh), hs(gf, hh),
                                        hs(Cf, hh), ALU.mult)
                nc.vector.tensor_tensor(hs(Cf, hh), hs(Cf, hh),
                                        hs(tmp, hh), ALU.add)
                nc.scalar.activation(hs(tcl, hh), hs(Cf, hh), AF.Tanh)
                nc.vector.tensor_tensor(hs(Hf, hh), hs(go, hh),
                                        hs(tcl, hh), ALU.mult)

        # ===== Phase 4: backward LSTM single step (only hb[:,0] used) =====
        Hb = lp.tile([128, 2 * ROWS], F16, name="Hb")
        kb = (WIN - 2) // 2
        for mt in [0, 1, 4, 5, 6, 7]:          # forget gate irrelevant (c0=0)
            ps = ps_b.tile([128, 1024], F32, tag="lpsb", name="lpsb")
            for hh in range(2):
                pslice = ps[:, hh * 512:hh * 512 + HROWS]
                b0 = hh * (BL // 2)
                for kt in range(2):
                    fv = Fts[kt][:].rearrange("p (b k two) -> p b k two",
                                              b=BL, two=2)
                    nc.tensor.matmul(
                        pslice, lxb[kt][:, mt * 128:(mt + 1) * 128],
                        fv[:, b0:b0 + BL // 2, kb:kb + NW, 1],
                        start=(kt == 0), stop=(kt == 1))
            dst = gate_dst[mt][:, (mt % 2) * ROWS:(mt % 2 + 1) * ROWS]
            dstv = dst.rearrange("p (h r) -> p h r", h=2)
            psv = ps[:].rearrange("p (h x) -> p h x", h=2)[:, :, 0:HROWS]
            fn = AF.Sigmoid if mt < 6 else AF.Tanh
            nc.scalar.activation(dstv, psv, fn,
                                 bias=bgb[:, mt:mt + 1], scale=1.0)
        nc.vector.tensor_tensor(tmp[:], gi[:], tg[:], ALU.mult)
        nc.scalar.activation(tcl[:], tmp[:], AF.Tanh)
        nc.vector.tensor_tensor(Hb[:], go[:], tcl[:], ALU.mult)

        # ===== Phase 5: FC head =====
        ps = ps_b.tile([128, 1024], F32, tag="lpsb", name="lpsf")
        rhs4 = [Hf[:, 0:ROWS], Hf[:, ROWS:2 * ROWS],
                Hb[:, 0:ROWS], Hb[:, ROWS:2 * ROWS]]
        for hh in range(2):
            for qt in range(4):
                nc.tensor.matmul(ps[:, hh * 512:hh * 512 + HROWS],
                                 wfct[qt][:],
                                 rhs4[qt].rearrange("p (h r) -> p h r",
                                                    h=2)[:, hh, :],
                                 start=(qt == 0), stop=(qt == 3))
        ob = lp.tile([EMB, ROWS], F32, name="ob")
        obv = ob[:].rearrange("p (h r) -> p h r", h=2)
        psv = ps[:].rearrange("p (h x) -> p h x", h=2)[:, :, 0:HROWS]
        nc.scalar.activation(obv, psv, AF.Identity,
                             bias=bfc[:, 0:1], scale=1.0)
        nc.sync.dma_start(io["out_d"][:], ob[:])

    ctx.close()


def _build_program():
    nc = bacc.Bacc("TRN2", target_bir_lowering=False, debug=False,
                   num_devices=NCORES)

    def din(name, shape, dt=F16):
        return nc.dram_tensor(name, shape, dt, kind="ExternalInput").ap()

    io = dict(
        x0A=din("x0A", [NCH, FIN, CH_FREE]),
        mixM=din("mixM", [NB, 128]),
        mixMb1=din("mixMb1", [NB + 1, 128]),
        mixMb3=din("mixMb3", [NB + 1, 128]),
        b1row=din("b1row", [1, GBLK * 64]),
        b3row=din("b3row", [1, GBLK * 256]),
        w1=din("w1", [FIN, 64]), w2=din("w2", [64, 128]),
        w3=din("w3", [128, 256]), w4=din("w4", [256, 256]),
        b2=din("b2", [128, 1], F32), b4=din("b4", [128, 2], F32),
        lxf=din("lxf", [256, 1024]), lhf=din("lhf", [256, 1024]),
        lxb=din("lxb", [256, 1024]),
        bgf=din("bgf", [128, 8], F32), bgb=din("bgb", [128, 8], F32),
        wfc=din("wfc", [512, 128]), bfc=din("bfc", [128, 1], F32),
        ident=din("ident", [128, 128]),
        out_d=nc.dram_tensor("out", [EMB, ROWS], F32,
                             kind="ExternalOutput").ap(),
    )
    with tile.TileContext(nc) as tc:
        _kernel_body(tc, io)
    nc.compile()
    return nc


def _host_prep(inputs):
    f16 = np.float16
    data = np.asarray(inputs["data"], np.float32)
    ei = np.asarray(inputs["edge_index"]).astype(np.int64)

    src = np.concatenate([ei[0], np.arange(N)])
    dst = np.concatenate([ei[1], np.arange(N)])
    deg = np.zeros(N, np.float32)
    np.add.at(deg, dst, 1.0)
    dinv = np.where(deg > 0, deg ** -0.5, 0.0).astype(np.float32)
    Ahat = np.zeros((N, N), np.float32)
    np.add.at(Ahat, (dst, src), dinv[src] * dinv[dst])
    mixM = np.kron(Ahat.T, np.eye(G5, dtype=np.float32)).astype(f16)  # [120,120]
    mixMp = np.zeros((NB, 128), f16)
    mixMp[:, 0:NB] = mixM

    def mixMb(bias_unused):
        m = np.zeros((NB + 1, 128), f16)
        m[0:NB, 0:NB] = mixM
        m[NB, 0:NB] = 1.0
        return m

    b1 = np.asarray(inputs["b1"], np.float32)
    b3 = np.asarray(inputs["b3"], np.float32)
    b1row = np.tile(b1[None, :], (GBLK, 1)).reshape(1, GBLK * 64).astype(f16)
    b3row = np.tile(b3[None, :], (GBLK, 1)).reshape(1, GBLK * 256).astype(f16)

    # x0A: [core][chunk b][c 6][gb*128 + blk], blk = n*5+g5, t = 5*gb+g5
    d = data.reshape(NCORES, BL, T, N, FIN)
    dpad = np.zeros((NCORES, BL, TP, N, FIN), np.float32)
    dpad[:, :, :T] = d
    dv = dpad.reshape(NCORES, BL, GBLK, G5, N, FIN)
    # -> [core, b, f, gb, n, g5]
    dv = dv.transpose(0, 1, 5, 2, 4, 3).reshape(NCORES, BL, FIN, GBLK, NB)
    x0A = np.zeros((NCORES, BL, FIN, GBLK, 128), np.float32)
    x0A[:, :, :, :, 0:NB] = dv
    x0A = np.ascontiguousarray(
        x0A.reshape(NCORES, BL, FIN, CH_FREE)).astype(f16)

    perm = np.concatenate([np.arange(0, H), np.arange(H, 2 * H),
                           np.arange(3 * H, 4 * H), np.arange(2 * H, 3 * H)])

    def prep_dir(wih, whh, bih, bhh):
        wihp = np.asarray(wih, np.float32)[perm] / N
        whhp = np.asarray(whh, np.float32)[perm]
        bg = (np.asarray(bih, np.float32) + np.asarray(bhh, np.float32))[perm]
        return (np.ascontiguousarray(wihp.T).astype(f16),
                np.ascontiguousarray(whhp.T).astype(f16),
                np.ascontiguousarray(bg.reshape(8, 128).T).astype(np.float32))

    lxf, lhf, bgf = prep_dir(inputs["lstm_Wih_f"], inputs["lstm_Whh_f"],
                             inputs["lstm_bih_f"], inputs["lstm_bhh_f"])
    lxb, _lhb, bgb = prep_dir(inputs["lstm_Wih_b"], inputs["lstm_Whh_b"],
                              inputs["lstm_bih_b"], inputs["lstm_bhh_b"])

    com = {
        "mixM": mixMp,
        "mixMb1": mixMb(None),
        "mixMb3": mixMb(None),
        "b1row": b1row,
        "b3row": b3row,
        "w1": np.asarray(inputs["W1"], np.float32).astype(f16),
        "w2": np.asarray(inputs["W2"], np.float32).astype(f16),
        "w3": np.asarray(inputs["W3"], np.float32).astype(f16),
        "w4": np.asarray(inputs["W4"], np.float32).astype(f16),
        "b2": np.asarray(inputs["b2"], np.float32).reshape(128, 1),
        "b4": np.ascontiguousarray(
            np.asarray(inputs["b4"], np.float32).reshape(2, 128).T),
        "lxf": lxf, "lhf": lhf, "lxb": lxb, "bgf": bgf, "bgb": bgb,
        "wfc": np.asarray(inputs["Wfc"], np.float32).astype(f16),
        "bfc": np.asarray(inputs["bfc"], np.float32).reshape(128, 1),
        "ident": np.eye(128, dtype=f16),
    }
    return [dict(com, x0A=x0A[c]) for c in range(NCORES)]


TRACE = False          # set by test harness to capture an NTFF profile


def kernel(**inputs) -> np.ndarray:
    if "nc" not in _CACHE:
        _CACHE["nc"] = _build_program()
    nc = _CACHE["nc"]
    in_maps = _host_prep(inputs)
    res = bass_utils.run_bass_kernel_spmd(nc, in_maps,
                                          core_ids=list(range(NCORES)),
                                          trace=TRACE)
    _CACHE["last_res"] = res
    outs = []
    for c in range(NCORES):
        o = res.results[c]["out"]                       # [128, 904]
        outs.append(o.reshape(EMB, BL, NW).transpose(1, 2, 0))
    return np.concatenate(outs, 0).astype(np.float32)   # [64, 113, 128]


if __name__ == "__main__":
    import reference
    ins = {k: np.asarray(v) for k, v in reference.setup_inputs().items()}
    out = kernel(**ins)
    print("kernel out", out.shape, out.dtype, float(np.abs(out).max()))


# revision 25
# speedup vs baseline: 2.2101x; 1.0087x over previous
"""Trainium2 Bass kernel for DeepConvGraphEncoderDownstream.

Model (per reference):
  4-layer GCN (shared dense 24x24 graph operator) applied per (batch, timestep)
  frame -> node-mean -> per sliding window (W=32, stride 2, 113 windows):
  BiLSTM(H=256) -> concat(h_fwd[-1], h_bwd[0]) @ Wfc + bfc.

Key algebraic restructurings:
  * gcn_norm folded into one dense Ahat[24,24] on host.
  * GCN runs ONCE over all 256 timesteps (the reference recomputes it ~14x
    across overlapping windows).
  * backward LSTM: only hb[:, 0] is used => exactly ONE step, no recurrence.
  * forward LSTM: all 113 windows batched into one 904-row recurrence per
    core; input transforms U precomputed per-chunk during the GCN phase.

Sharding: data-parallel over batch, 8 batches/core on 8 cores; output
slices are independent (no collectives).

GCN layout scheme (NO DMA transposes — the v1 kernel spent ~570us/core in
serialized XBAR DMA_TRANSPOSE ops):
  A-layout [c_part, free=(gb:52, blk:128)], blk = n*5+g5, t = 5*gb+g5.
  B-layout [blk_part, free=(gb:52, c)].
  Per GCN layer, one matmul runs "data-as-stationary" (lhsT = activation
  tile block, rhs = small operator) which flips layout A<->B as a side
  effect of out = lhsT.T @ rhs; the other matmul runs classic (operator
  stationary, activations moving).  Layer pairing:
    L1: transform-ds (A->B) + mix-classic  (B->B)   relu/bias via mix row
    L2: mix-ds      (B->A) + transform-cl (A->A)   relu+bias at eviction
    L3: transform-ds (A->B) + mix-classic  (B->B)   relu/bias via mix row
    L4: mix-ds      (B->A) + transform-cl (A->A)   relu+bias at eviction
  B-side bias: stationary mix matrix has an extra all-ones row 120 that
  multiplies a bias pattern pre-written into partition 120 of the B tile.
  Node-mean after L4: in-place DVE tree-sum over the n-stride-5 free dim,
  last add writes directly into the F tile.
PSUM evictions are spread across Vector/Scalar(Act)/GpSimd(Pool) engines.
"""

import os
import sys
import numpy as np

try:
    import concourse.bass as bass
except ImportError:
    sys.path.insert(0, "/opt/trn_rl_repo")
    import concourse.bass as bass

import concourse.bacc as bacc
import concourse.tile as tile
from concourse import mybir
from concourse import bass_utils

F16 = mybir.dt.float16
F32 = mybir.dt.float32
AF = mybir.ActivationFunctionType
ALU = mybir.AluOpType

B, T, N, FIN = 64, 256, 24, 6
H, EMB = 256, 128
WIN = 32
NW = (T - WIN) // 2 + 1               # 113
NCORES = 8
BL = B // NCORES                      # 8
G5 = 5
GBLK = 52                             # 52*5 = 260 t-slots
TP = GBLK * G5                        # 260 padded timesteps
NCH = BL
ROWS = BL * NW                        # 904
HROWS = ROWS // 2                     # 452
CH_FREE = GBLK * 128                  # 6656 A-layout free per chunk
FTOT = BL * TP                        # 2080 F columns
NB = N * G5                           # 120 valid blk rows

_CACHE = {}


def _kernel_body(tc, io):
    nc = tc.nc
    from contextlib import ExitStack
    ctx = ExitStack()

    cons = ctx.enter_context(tc.tile_pool(name="cons", bufs=1))
    fpool = ctx.enter_context(tc.tile_pool(name="fpool", bufs=1))

    def load_const(name, shape, dt=F16):
        t = cons.tile(shape, dt, name=name)
        nc.sync.dma_start(t[:], io[name][:])
        return t

    mixM = load_const("mixM", [NB, 128])          # plain kron(Ahat^T,I5)
    mixMb1 = load_const("mixMb1", [NB + 1, 128])  # + bias row 120
    mixMb3 = load_const("mixMb3", [NB + 1, 128])
    w1 = load_const("w1", [FIN, 64])
    w2d = load_const("w2d", [128, 128])   # [W2; W2] stacked for parity sweeps
    w3 = load_const("w3", [128, 256])
    b2 = load_const("b2", [128, 1], F32)
    b4 = load_const("b4", [128, 2], F32)
    w4k = []
    for kt in range(2):
        t = cons.tile([128, 256], F16, name=f"w4k{kt}")
        nc.sync.dma_start(t[:], io["w4"][kt * 128:(kt + 1) * 128, :])
        w4k.append(t)

    def load_ktiles(pool, name):
        ts = []
        for kt in range(2):
            t = pool.tile([128, 1024], F16, name=f"{name}{kt}")
            nc.sync.dma_start(t[:], io[name][kt * 128:(kt + 1) * 128, :])
            ts.append(t)
        return ts

    lxf = load_ktiles(cons, "lxf")

    F0 = fpool.tile([128, FTOT], F16, name="F0")
    F1 = fpool.tile([128, FTOT], F16, name="F1")
    Fts = [F0, F1]
    Umt = [fpool.tile([128, FTOT], F16, name=f"U{mt}") for mt in range(8)]

    # ================= Phase 1: GCN + per-chunk U =================
    with tc.tile_pool(name="gx0", bufs=1) as gx0, \
         tc.tile_pool(name="gwork", bufs=1) as gw, \
         tc.tile_pool(name="psA", bufs=2, space="PSUM") as psA, \
         tc.tile_pool(name="psB", bufs=2, space="PSUM") as psB:

        # persistent per-stage work tiles (reused across chunks)
        B1 = gw.tile([NB + 1, GBLK * 64], F16, name="B1")
        B1p = gw.tile([NB, GBLK * 64], F16, name="B1p")
        # A2s: gb-PAIR stacked layout [64*(gb%2)+c, ((gb//2), blk')]
        A2s = gw.tile([128, 26 * 128], F16, name="A2s")
        A2p = [gw.tile([128, 26 * 128], F16, name=f"A2p_{par}")
               for par in range(2)]        # by gb parity
        # B3 is 128-part so the t-cl4 outputs X4 can alias its storage
        # (B3's last read, mix-cl3, precedes the first X4 write in PE order)
        B3 = gw.tile([128, GBLK * 256], F16, name="B3")
        B3p = gw.tile([NB, GBLK * 256], F16, name="B3p")
        A4 = [gw.tile([128, CH_FREE], F16, name=f"A4_{h}") for h in range(2)]
        X4 = [B3[:, m * CH_FREE:(m + 1) * CH_FREE] for m in range(2)]

        # bias pattern into row 120 of B1 (once; B1 rows 0:120 only are
        # rewritten per chunk).  B3's row 120 is re-loaded per chunk since
        # the aliased X4 writes clobber it.
        nc.sync.dma_start(B1[NB:NB + 1, :], io["b1row"][:])

        def emit_U(k):
            # U_k = F_k @ (Wih_f/24)^T — emitted one chunk late so the
            # tree-sum feeding F_k has drained off the vector engine
            for mp in range(4):
                ps = psA.tile([128, 1024], F32, tag="psA", name="psu")
                for sub in range(2):
                    mt = mp * 2 + sub
                    for kt in range(2):
                        nc.tensor.matmul(ps[:, sub * 512:sub * 512 + TP],
                                         lxf[kt][:, mt * 128:(mt + 1) * 128],
                                         Fts[kt][:, k * TP:(k + 1) * TP],
                                         start=(kt == 0), stop=(kt == 1))
                for sub in range(2):
                    mt = mp * 2 + sub
                    dst = Umt[mt][:, k * TP:(k + 1) * TP]
                    src = ps[:, sub * 512:sub * 512 + TP]
                    if mt % 2:
                        nc.scalar.copy(dst, src)
                    else:
                        nc.vector.tensor_copy(dst, src)

        for k in range(NCH):
            x0 = gx0.tile([FIN, CH_FREE], F16, tag="x0", name="x0")
            nc.sync.dma_start(x0[:], io["x0A"][k])
            nc.sync.dma_start(B3[NB:NB + 1, :], io["b3row"][:])

            # --- L1 transform-ds: per gb, out_B[blk, c'64] = x0_gb.T @ W1
            for bk in range(4):             # psum tiles of 16 gb (last 4)
                ng = min(16, 52 - bk * 16)
                ps = psA.tile([128, 1024], F32, tag="psA", name="ps1")
                for g in range(ng):
                    gb = bk * 16 + g
                    nc.tensor.matmul(ps[:, g * 64:(g + 1) * 64],
                                     x0[:, gb * 128:(gb + 1) * 128],
                                     w1[:],
                                     start=True, stop=True)
                nc.vector.tensor_copy(B1[0:NB, bk * 1024:bk * 1024 + ng * 64],
                                      ps[0:NB, 0:ng * 64])

            # --- L1 mix-classic (+bias row +relu): B1 -> B1p
            for fc in range(4):             # 3328 = 3*1024 + 256
                f0 = fc * 1024
                fw = min(1024, GBLK * 64 - f0)
                ps = psB.tile([128, 1024], F32, tag="psB", name="ps1m")
                for g0 in range(0, fw, 512):
                    gw_ = min(512, fw - g0)
                    nc.tensor.matmul(ps[:, g0:g0 + gw_], mixMb1[:],
                                     B1[:, f0 + g0:f0 + g0 + gw_],
                                     start=True, stop=True)
                nc.scalar.activation(B1p[:, f0:f0 + fw], ps[0:NB, 0:fw],
                                     AF.Relu)

            # --- L2 mix-ds (gb pairs): out[64*(gb%2)+c, blk'] = B1p_pair.T @ mixM
            for bk in range(4):             # psum tiles of 8 pairs (last 2)
                npr = min(8, 26 - bk * 8)
                ps = psA.tile([128, 1024], F32, tag="psA", name="ps2")
                for g in range(npr):
                    gp = bk * 8 + g
                    nc.tensor.matmul(ps[:, g * 128:(g + 1) * 128],
                                     B1p[:, gp * 128:(gp + 1) * 128],
                                     mixM[:],
                                     start=True, stop=True)
                nc.vector.tensor_copy(
                    A2s[:, bk * 1024:bk * 1024 + npr * 128],
                    ps[:, 0:npr * 128])

            # --- L2 transform-cl (+bias+relu): A2s -> A2p[parity]
            for par in range(2):
                for fc in range(4):         # 3328 = 3*1024 + 256
                    f0 = fc * 1024
                    fw = min(1024, 26 * 128 - f0)
                    ps = psB.tile([128, 1024], F32, tag="psB", name="ps2t")
                    for g0 in range(0, fw, 512):
                        gw_ = min(512, fw - g0)
                        nc.tensor.matmul(
                            ps[:, g0:g0 + gw_],
                            w2d[par * 64:(par + 1) * 64, :],
                            A2s[par * 64:(par + 1) * 64,
                                f0 + g0:f0 + g0 + gw_],
                            start=True, stop=True)
                    nc.scalar.activation(A2p[par][:, f0:f0 + fw],
                                         ps[:, 0:fw], AF.Relu,
                                         bias=b2[:, 0:1], scale=1.0)

            if k > 0:
                emit_U(k - 1)

            # --- L3 transform-ds: per gb, out_B[blk, c'256] = A2p_gb.T @ W3
            for bk in range(13):            # psum tiles of 4 gb
                ps = psA.tile([128, 1024], F32, tag="psA", name="ps3")
                for g in range(4):
                    gb = bk * 4 + g
                    nc.tensor.matmul(ps[:, g * 256:(g + 1) * 256],
                                     A2p[gb % 2][:, (gb // 2) * 128:
                                                 (gb // 2 + 1) * 128],
                                     w3[:],
                                     start=True, stop=True)
                nc.vector.tensor_copy(B3[0:NB, bk * 1024:(bk + 1) * 1024],
                                      ps[0:NB, :])

            # --- L3 mix-classic (+bias row +relu): B3 -> B3p
            for fc in range(13):
                f0 = fc * 1024
                ps = psB.tile([128, 1024], F32, tag="psB", name="ps3m")
                for g0 in (0, 512):
                    nc.tensor.matmul(ps[:, g0:g0 + 512], mixMb3[:],
                                     B3[0:NB + 1, f0 + g0:f0 + g0 + 512],
                                     start=True, stop=True)
                nc.scalar.activation(B3p[:, f0:f0 + 1024], ps[0:NB, :],
                                     AF.Relu)

            # --- L4 mix-ds: per (gb, c-half), out_A[c128, blk'] = B3p_gb_h.T @ mixM
            for h in range(2):
                for bk in range(7):         # psum tiles of 8 gb (last 4)
                    ng = min(8, 52 - bk * 8)
                    ps = psA.tile([128, 1024], F32, tag="psA", name="ps4")
                    for g in range(ng):
                        gb = bk * 8 + g
                        c0 = gb * 256 + h * 128
                        nc.tensor.matmul(ps[:, g * 128:(g + 1) * 128],
                                         B3p[:, c0:c0 + 128],
                                         mixM[:],
                                         start=True, stop=True)
                    dst = A4[h][:, bk * 1024:bk * 1024 + ng * 128]
                    if bk % 2 == 0:
                        nc.scalar.copy(dst, ps[:, 0:ng * 128])
                    else:
                        nc.vector.tensor_copy(dst, ps[:, 0:ng * 128])

            # --- L4 transform-cl (+bias+relu): A4 -> X4 (2 m-tiles)
            for mt in range(2):
                for fc in range(7):         # 6656 = 6*1024 + 512
                    f0 = fc * 1024
                    fw = min(1024, CH_FREE - f0)
                    ps = psB.tile([128, 1024], F32, tag="psB", name="ps4t")
                    for g0 in range(0, fw, 512):
                        for kt in range(2):
                            nc.tensor.matmul(
                                ps[:, g0:g0 + 512],
                                w4k[kt][:, mt * 128:(mt + 1) * 128],
                                A4[kt][:, f0 + g0:f0 + g0 + 512],
                                start=(kt == 0), stop=(kt == 1))
                    nc.scalar.activation(X4[mt][:, f0:f0 + fw],
                                         ps[:, 0:fw],
                                         AF.Relu, bias=b4[:, mt:mt + 1],
                                         scale=1.0)

            # --- node-sum tree (24 nodes, stride 5 in blk) -> Fts chunk k
            for mt in range(2):
                xv = X4[mt][:].rearrange("p (gb blk) -> p gb blk", blk=128)
                nc.vector.tensor_tensor(xv[:, :, 0:60], xv[:, :, 0:60],
                                        xv[:, :, 60:120], ALU.add)
                nc.vector.tensor_tensor(xv[:, :, 0:30], xv[:, :, 0:30],
                                        xv[:, :, 30:60], ALU.add)
                nc.vector.tensor_tensor(xv[:, :, 0:15], xv[:, :, 0:15],
                                        xv[:, :, 15:30], ALU.add)
                nc.vector.tensor_tensor(xv[:, :, 0:5], xv[:, :, 0:5],
                                        xv[:, :, 5:10], ALU.add)
                dstv = Fts[mt][:, k * TP:(k + 1) * TP].rearrange(
                    "p (gb g5) -> p gb g5", g5=G5)
                nc.vector.tensor_tensor(dstv, xv[:, :, 0:5],
                                        xv[:, :, 10:15], ALU.add)

        emit_U(NCH - 1)

    # ===== LSTM-only constants (loaded after the GCN pools free SBUF) =====
    cons2 = ctx.enter_context(tc.tile_pool(name="cons2", bufs=1))
    lhf = load_ktiles(cons2, "lhf")
    lxb = load_ktiles(cons2, "lxb")

    def load_const2(name, shape, dt=F16):
        t = cons2.tile(shape, dt, name=name)
        nc.sync.dma_start(t[:], io[name][:])
        return t

    bgf = load_const2("bgf", [128, 8], F32)
    bgb = load_const2("bgb", [128, 8], F32)
    ident = load_const2("ident", [128, 128])
    wfct = []
    for qt in range(4):
        t = cons2.tile([128, 128], F16, name=f"wfct{qt}")
        nc.sync.dma_start(t[:], io["wfc"][qt * 128:(qt + 1) * 128, :])
        wfct.append(t)
    bfc = load_const2("bfc", [128, 1], F32)

    # ================= Phase 3: forward LSTM =================
    lp = ctx.enter_context(tc.tile_pool(name="lstm", bufs=1))
    Hf = lp.tile([128, 2 * ROWS], F16, name="Hf")
    Cf = lp.tile([128, 2 * ROWS], F16, name="Cf")
    nc.vector.memset(Hf[:], 0.0)
    nc.vector.memset(Cf[:], 0.0)
    gi = lp.tile([128, 2 * ROWS], F16, name="gi")
    gf = lp.tile([128, 2 * ROWS], F16, name="gf")
    go = lp.tile([128, 2 * ROWS], F16, name="go")
    tg = lp.tile([128, 2 * ROWS], F16, name="tg")
    tcl = lp.tile([128, 2 * ROWS], F16, name="tcl")
    tmp = lp.tile([128, 2 * ROWS], F16, name="tmp")
    gate_dst = [gi, gi, gf, gf, go, go, tg, tg]

    def hs(t, hh):
        # [128, (g:2, hh:2, 452)] -> the hh half across both 904-col groups
        return t[:].rearrange("p (g h r) -> p g h r", g=2,
                              r=HROWS)[:, :, hh, :]

    with tc.tile_pool(name="lps", bufs=4, space="PSUM") as ps_l, \
         tc.tile_pool(name="lpsb", bufs=2, space="PSUM") as ps_b:
        # two independent row-half recurrences, software-pipelined so the
        # activation/combine tail of one half hides under the other's matmuls
        for s in range(WIN):
            k0, par = s // 2, s % 2
            for hh in range(2):
                b0 = hh * (BL // 2)
                for mt in range(8):
                    ps = ps_l.tile([128, 512], F32, tag="lps", name="lps")
                    uv = Umt[mt][:].rearrange("p (b k two) -> p b k two",
                                              b=BL, two=2)
                    nc.tensor.matmul(
                        ps[:, 0:HROWS], ident[:],
                        uv[:, b0:b0 + BL // 2, k0:k0 + NW, par],
                        start=True, stop=False)
                    for kt in range(2):
                        nc.tensor.matmul(
                            ps[:, 0:HROWS],
                            lhf[kt][:, mt * 128:(mt + 1) * 128],
                            Hf[:, kt * ROWS + hh * HROWS:
                               kt * ROWS + (hh + 1) * HROWS],
                            start=False, stop=(kt == 1))
                    dst = gate_dst[mt][:, (mt % 2) * ROWS + hh * HROWS:
                                       (mt % 2) * ROWS + (hh + 1) * HROWS]
                    fn = AF.Sigmoid if mt < 6 else AF.Tanh
                    nc.scalar.activation(dst, ps[:, 0:HROWS], fn,
                                         bias=bgf[:, mt:mt + 1], scale=1.0)
                nc.vector.tensor_tensor(hs(tmp, hh), hs(gi, hh),
                                        hs(tg, hh), ALU.mult)
                nc.vector.tensor_tensor(hs(Cf, hh), hs(gf, hh),
                                        hs(Cf, hh), ALU.mult)
                nc.vector.tensor_tensor(hs(Cf, hh), hs(Cf, hh),
                                        hs(tmp, hh), ALU.add)
                nc.scalar.activation(hs(tcl, hh), hs(Cf, hh), AF.Tanh)
                nc.vector.tensor_tensor(hs(Hf, hh), hs(go, hh),
                                        hs(tcl, hh), ALU.mult)

        # ===== Phase 4: backward LSTM single step (only hb[:,0] used) =====
        Hb = lp.tile([128, 2 * ROWS], F16, name="Hb")
        kb = (WIN - 2) // 2
        for mt in [0, 1, 4, 5, 6, 7]:          # forget gate irrelevant (c0=0)
            ps = ps_b.tile([128, 1024], F32, tag="lpsb", name="lpsb")
            for hh in range(2):
                pslice = ps[:, hh * 512:hh * 512 + HROWS]
                b0 = hh * (BL // 2)
                for kt in range(2):
                    fv = Fts[kt][:].rearrange("p (b k two) -> p b k two",
                                              b=BL, two=2)
                    nc.tensor.matmul(
                        pslice, lxb[kt][:, mt * 128:(mt + 1) * 128],
                        fv[:, b0:b0 + BL // 2, kb:kb + NW, 1],
                        start=(kt == 0), stop=(kt == 1))
            dst = gate_dst[mt][:, (mt % 2) * ROWS:(mt % 2 + 1) * ROWS]
            dstv = dst.rearrange("p (h r) -> p h r", h=2)
            psv = ps[:].rearrange("p (h x) -> p h x", h=2)[:, :, 0:HROWS]
            fn = AF.Sigmoid if mt < 6 else AF.Tanh
            nc.scalar.activation(dstv, psv, fn,
                                 bias=bgb[:, mt:mt + 1], scale=1.0)
        nc.vector.tensor_tensor(tmp[:], gi[:], tg[:], ALU.mult)
        nc.scalar.activation(tcl[:], tmp[:], AF.Tanh)
        nc.vector.tensor_tensor(Hb[:], go[:], tcl[:], ALU.mult)

        # ===== Phase 5: FC head =====
        ps = ps_b.tile([128, 1024], F32, tag="lpsb", name="lpsf")
        rhs4 = [Hf[:, 0:ROWS], Hf[:, ROWS:2 * ROWS],
                Hb[:, 0:ROWS], Hb[:, ROWS:2 * ROWS]]
        for hh in range(2):
            for qt in range(4):
                nc.tensor.matmul(ps[:, hh * 512:hh * 512 + HROWS],
                                 wfct[qt][:],
                                 rhs4[qt].rearrange("p (h r) -> p h r",
                                                    h=2)[:, hh, :],
                                 start=(qt == 0), stop=(qt == 3))
        ob = lp.tile([EMB, ROWS], F32, name="ob")
        obv = ob[:].rearrange("p (h r) -> p h r", h=2)
        psv = ps[:].rearrange("p (h x) -> p h x", h=2)[:, :, 0:HROWS]
        nc.scalar.activation(obv, psv, AF.Identity,
                             bias=bfc[:, 0:1], scale=1.0)
        nc.sync.dma_start(io["out_d"][:], ob[:])

    ctx.close()


def _build_program():
    nc = bacc.Bacc("TRN2", target_bir_lowering=False, debug=False,
                   num_devices=NCORES)

    def din(name, shape, dt=F16):
        return nc.dram_tensor(name, shape, dt, kind="ExternalInput").ap()

    io = dict(
        x0A=din("x0A", [NCH, FIN, CH_FREE]),
        mixM=din("mixM", [NB, 128]),
        mixMb1=din("mixMb1", [NB + 1, 128]),
        mixMb3=din("mixMb3", [NB + 1, 128]),
        b1row=din("b1row", [1, GBLK * 64]),
        b3row=din("b3row", [1, GBLK * 256]),
        w1=din("w1", [FIN, 64]), w2d=din("w2d", [128, 128]),
        w3=din("w3", [128, 256]), w4=din("w4", [256, 256]),
        b2=din("b2", [128, 1], F32), b4=din("b4", [128, 2], F32),
        lxf=din("lxf", [256, 1024]), lhf=din("lhf", [256, 1024]),
        lxb=din("lxb", [256, 1024]),
        bgf=din("bgf", [128, 8], F32), bgb=din("bgb", [128, 8], F32),
        wfc=din("wfc", [512, 128]), bfc=din("bfc", [128, 1], F32),
        ident=din("ident", [128, 128]),
        out_d=nc.dram_tensor("out", [EMB, ROWS], F32,
                             kind="ExternalOutput").ap(),
    )
    with tile.TileContext(nc) as tc:
        _kernel_body(tc, io)
    nc.compile()
    return nc


def _host_prep(inputs):
    f16 = np.float16
    data = np.asarray(inputs["data"], np.float32)
    ei = np.asarray(inputs["edge_index"]).astype(np.int64)

    src = np.concatenate([ei[0], np.arange(N)])
    dst = np.concatenate([ei[1], np.arange(N)])
    deg = np.zeros(N, np.float32)
    np.add.at(deg, dst, 1.0)
    dinv = np.where(deg > 0, deg ** -0.5, 0.0).astype(np.float32)
    Ahat = np.zeros((N, N), np.float32)
    np.add.at(Ahat, (dst, src), dinv[src] * dinv[dst])
    mixM = np.kron(Ahat.T, np.eye(G5, dtype=np.float32)).astype(f16)  # [120,120]
    mixMp = np.zeros((NB, 128), f16)
    mixMp[:, 0:NB] = mixM

    def mixMb(bias_unused):
        m = np.zeros((NB + 1, 128), f16)
        m[0:NB, 0:NB] = mixM
        m[NB, 0:NB] = 1.0
        return m

    b1 = np.asarray(inputs["b1"], np.float32)
    b3 = np.asarray(inputs["b3"], np.float32)
    b1row = np.tile(b1[None, :], (GBLK, 1)).reshape(1, GBLK * 64).astype(f16)
    b3row = np.tile(b3[None, :], (GBLK, 1)).reshape(1, GBLK * 256).astype(f16)

    # x0A: [core][chunk b][c 6][gb*128 + blk], blk = n*5+g5, t = 5*gb+g5
    d = data.reshape(NCORES, BL, T, N, FIN)
    dpad = np.zeros((NCORES, BL, TP, N, FIN), np.float32)
    dpad[:, :, :T] = d
    dv = dpad.reshape(NCORES, BL, GBLK, G5, N, FIN)
    # -> [core, b, f, gb, n, g5]
    dv = dv.transpose(0, 1, 5, 2, 4, 3).reshape(NCORES, BL, FIN, GBLK, NB)
    x0A = np.zeros((NCORES, BL, FIN, GBLK, 128), np.float32)
    x0A[:, :, :, :, 0:NB] = dv
    x0A = np.ascontiguousarray(
        x0A.reshape(NCORES, BL, FIN, CH_FREE)).astype(f16)

    perm = np.concatenate([np.arange(0, H), np.arange(H, 2 * H),
                           np.arange(3 * H, 4 * H), np.arange(2 * H, 3 * H)])

    def prep_dir(wih, whh, bih, bhh):
        wihp = np.asarray(wih, np.float32)[perm] / N
        whhp = np.asarray(whh, np.float32)[perm]
        bg = (np.asarray(bih, np.float32) + np.asarray(bhh, np.float32))[perm]
        return (np.ascontiguousarray(wihp.T).astype(f16),
                np.ascontiguousarray(whhp.T).astype(f16),
                np.ascontiguousarray(bg.reshape(8, 128).T).astype(np.float32))

    lxf, lhf, bgf = prep_dir(inputs["lstm_Wih_f"], inputs["lstm_Whh_f"],
                             inputs["lstm_bih_f"], inputs["lstm_bhh_f"])
    lxb, _lhb, bgb = prep_dir(inputs["lstm_Wih_b"], inputs["lstm_Whh_b"],
                              inputs["lstm_bih_b"], inputs["lstm_bhh_b"])

    com = {
        "mixM": mixMp,
        "mixMb1": mixMb(None),
        "mixMb3": mixMb(None),
        "b1row": b1row,
        "b3row": b3row,
        "w1": np.asarray(inputs["W1"], np.float32).astype(f16),
        "w2d": np.concatenate([np.asarray(inputs["W2"], np.float32)] * 2,
                              axis=0).astype(f16),
        "w3": np.asarray(inputs["W3"], np.float32).astype(f16),
        "w4": np.asarray(inputs["W4"], np.float32).astype(f16),
        "b2": np.asarray(inputs["b2"], np.float32).reshape(128, 1),
        "b4": np.ascontiguousarray(
            np.asarray(inputs["b4"], np.float32).reshape(2, 128).T),
        "lxf": lxf, "lhf": lhf, "lxb": lxb, "bgf": bgf, "bgb": bgb,
        "wfc": np.asarray(inputs["Wfc"], np.float32).astype(f16),
        "bfc": np.asarray(inputs["bfc"], np.float32).reshape(128, 1),
        "ident": np.eye(128, dtype=f16),
    }
    return [dict(com, x0A=x0A[c]) for c in range(NCORES)]


TRACE = False          # set by test harness to capture an NTFF profile


def kernel(**inputs) -> np.ndarray:
    if "nc" not in _CACHE:
        _CACHE["nc"] = _build_program()
    nc = _CACHE["nc"]
    in_maps = _host_prep(inputs)
    res = bass_utils.run_bass_kernel_spmd(nc, in_maps,
                                          core_ids=list(range(NCORES)),
                                          trace=TRACE)
    _CACHE["last_res"] = res
    outs = []
    for c in range(NCORES):
        o = res.results[c]["out"]                       # [128, 904]
        outs.append(o.reshape(EMB, BL, NW).transpose(1, 2, 0))
    return np.concatenate(outs, 0).astype(np.float32)   # [64, 113, 128]


if __name__ == "__main__":
    import reference
    ins = {k: np.asarray(v) for k, v in reference.setup_inputs().items()}
    out = kernel(**ins)
    print("kernel out", out.shape, out.dtype, float(np.abs(out).max()))


# revision 27
# speedup vs baseline: 2.2988x; 1.0401x over previous
"""Trainium2 Bass kernel for DeepConvGraphEncoderDownstream.

Model (per reference):
  4-layer GCN (shared dense 24x24 graph operator) applied per (batch, timestep)
  frame -> node-mean -> per sliding window (W=32, stride 2, 113 windows):
  BiLSTM(H=256) -> concat(h_fwd[-1], h_bwd[0]) @ Wfc + bfc.

Key algebraic restructurings:
  * gcn_norm folded into one dense Ahat[24,24] on host.
  * GCN runs ONCE over all 256 timesteps (the reference recomputes it ~14x
    across overlapping windows).
  * backward LSTM: only hb[:, 0] is used => exactly ONE step, no recurrence.
  * forward LSTM: all 113 windows batched into one 904-row recurrence per
    core; input transforms U precomputed per-chunk during the GCN phase.

Sharding: data-parallel over batch, 8 batches/core on 8 cores; output
slices are independent (no collectives).

GCN layout scheme (NO DMA transposes — the v1 kernel spent ~570us/core in
serialized XBAR DMA_TRANSPOSE ops):
  A-layout [c_part, free=(gb:52, blk:128)], blk = n*5+g5, t = 5*gb+g5.
  B-layout [blk_part, free=(gb:52, c)].
  Per GCN layer, one matmul runs "data-as-stationary" (lhsT = activation
  tile block, rhs = small operator) which flips layout A<->B as a side
  effect of out = lhsT.T @ rhs; the other matmul runs classic (operator
  stationary, activations moving).  Layer pairing:
    L1: transform-ds (A->B) + mix-classic  (B->B)   relu/bias via mix row
    L2: mix-ds      (B->A) + transform-cl (A->A)   relu+bias at eviction
    L3: transform-ds (A->B) + mix-classic  (B->B)   relu/bias via mix row
    L4: mix-ds      (B->A) + transform-cl (A->A)   relu+bias at eviction
  B-side bias: stationary mix matrix has an extra all-ones row 120 that
  multiplies a bias pattern pre-written into partition 120 of the B tile.
  Node-mean after L4: in-place DVE tree-sum over the n-stride-5 free dim,
  last add writes directly into the F tile.
PSUM evictions are spread across Vector/Scalar(Act)/GpSimd(Pool) engines.
"""

import os
import sys
import numpy as np

try:
    import concourse.bass as bass
except ImportError:
    sys.path.insert(0, "/opt/trn_rl_repo")
    import concourse.bass as bass

import concourse.bacc as bacc
import concourse.tile as tile
from concourse import mybir
from concourse import bass_utils

F16 = mybir.dt.float16
F32 = mybir.dt.float32
AF = mybir.ActivationFunctionType
ALU = mybir.AluOpType

B, T, N, FIN = 64, 256, 24, 6
H, EMB = 256, 128
WIN = 32
NW = (T - WIN) // 2 + 1               # 113
NCORES = 8
BL = B // NCORES                      # 8
G5 = 5
GBLK = 52                             # 52*5 = 260 t-slots
TP = GBLK * G5                        # 260 padded timesteps
NCH = BL
ROWS = BL * NW                        # 904
HROWS = ROWS // 2                     # 452
CH_FREE = GBLK * 128                  # 6656 A-layout free per chunk
FTOT = BL * TP                        # 2080 F columns
NB = N * G5                           # 120 valid blk rows

_CACHE = {}


def _kernel_body(tc, io):
    nc = tc.nc
    from contextlib import ExitStack
    ctx = ExitStack()

    cons = ctx.enter_context(tc.tile_pool(name="cons", bufs=1))
    fpool = ctx.enter_context(tc.tile_pool(name="fpool", bufs=1))

    def load_const(name, shape, dt=F16):
        t = cons.tile(shape, dt, name=name)
        nc.sync.dma_start(t[:], io[name][:])
        return t

    mixM = load_const("mixM", [NB, 128])          # plain kron(Ahat^T,I5)
    mixMb1 = load_const("mixMb1", [NB + 1, 128])  # + bias row 120
    mixMb3 = load_const("mixMb3", [NB + 1, 128])
    w1 = load_const("w1", [FIN, 64])
    w2d = load_const("w2d", [128, 128])   # [W2; W2] stacked for parity sweeps
    w3 = load_const("w3", [128, 256])
    b2 = load_const("b2", [128, 1], F32)
    b4 = load_const("b4", [128, 2], F32)
    w4k = []
    for kt in range(2):
        t = cons.tile([128, 256], F16, name=f"w4k{kt}")
        nc.sync.dma_start(t[:], io["w4"][kt * 128:(kt + 1) * 128, :])
        w4k.append(t)

    def load_ktiles(pool, name):
        ts = []
        for kt in range(2):
            t = pool.tile([128, 1024], F16, name=f"{name}{kt}")
            nc.sync.dma_start(t[:], io[name][kt * 128:(kt + 1) * 128, :])
            ts.append(t)
        return ts

    lxf = load_ktiles(cons, "lxf")

    F0 = fpool.tile([128, FTOT], F16, name="F0")
    F1 = fpool.tile([128, FTOT], F16, name="F1")
    Fts = [F0, F1]
    Umt = [fpool.tile([128, FTOT], F16, name=f"U{mt}") for mt in range(8)]

    # ================= Phase 1: GCN + per-chunk U =================
    with tc.tile_pool(name="gx0", bufs=1) as gx0, \
         tc.tile_pool(name="gwork", bufs=1) as gw, \
         tc.tile_pool(name="psA", bufs=2, space="PSUM") as psA, \
         tc.tile_pool(name="psB", bufs=2, space="PSUM") as psB:

        # persistent per-stage work tiles (reused across chunks)
        B1 = gw.tile([NB + 1, GBLK * 64], F16, name="B1")
        B1p = gw.tile([NB, GBLK * 64], F16, name="B1p")
        # A2s: gb-PAIR stacked layout [64*(gb%2)+c, ((gb//2), blk')]
        A2s = gw.tile([128, 26 * 128], F16, name="A2s")
        A2p = [gw.tile([128, 26 * 128], F16, name=f"A2p_{par}")
               for par in range(2)]        # by gb parity
        # B3 is 128-part so the t-cl4 outputs X4 can alias its storage
        # (B3's last read, mix-cl3, precedes the first X4 write in PE order)
        B3 = gw.tile([128, GBLK * 256], F16, name="B3")
        B3p = gw.tile([NB, GBLK * 256], F16, name="B3p")
        A4 = [gw.tile([128, CH_FREE], F16, name=f"A4_{h}") for h in range(2)]
        X4 = [B3[:, m * CH_FREE:(m + 1) * CH_FREE] for m in range(2)]

        # bias pattern into row 120 of B1 (once; B1 rows 0:120 only are
        # rewritten per chunk).  B3's row 120 is re-loaded per chunk since
        # the aliased X4 writes clobber it.
        nc.sync.dma_start(B1[NB:NB + 1, :], io["b1row"][:])

        def emit_U(k):
            # U_k = F_k @ (Wih_f/24)^T — emitted one chunk late so the
            # tree-sum feeding F_k has drained off the vector engine
            for mp in range(4):
                ps = psA.tile([128, 1024], F32, tag="psA", name="psu")
                for sub in range(2):
                    mt = mp * 2 + sub
                    for kt in range(2):
                        nc.tensor.matmul(ps[:, sub * 512:sub * 512 + TP],
                                         lxf[kt][:, mt * 128:(mt + 1) * 128],
                                         Fts[kt][:, k * TP:(k + 1) * TP],
                                         start=(kt == 0), stop=(kt == 1))
                for sub in range(2):
                    mt = mp * 2 + sub
                    dst = Umt[mt][:, k * TP:(k + 1) * TP]
                    src = ps[:, sub * 512:sub * 512 + TP]
                    if mt % 2:
                        nc.scalar.copy(dst, src)
                    else:
                        nc.vector.tensor_copy(dst, src)

        for k in range(NCH):
            x0 = gx0.tile([FIN, CH_FREE], F16, tag="x0", name="x0")
            nc.sync.dma_start(x0[:], io["x0A"][k])
            nc.sync.dma_start(B3[NB:NB + 1, :], io["b3row"][:])

            # Each layer interleaves its datastat stage (evict-heavy,
            # PE-light) with its classic stage (PE-heavy) at 1024-col tile
            # granularity, lag 1, so DVE/Act evictions hide under matmuls.

            def tds1(bk):
                ng = min(16, 52 - bk * 16)
                ps = psA.tile([128, 1024], F32, tag="psA", name="ps1")
                for g in range(ng):
                    gb = bk * 16 + g
                    nc.tensor.matmul(ps[:, g * 64:(g + 1) * 64],
                                     x0[:, gb * 128:(gb + 1) * 128],
                                     w1[:],
                                     start=True, stop=True)
                nc.vector.tensor_copy(B1[0:NB, bk * 1024:bk * 1024 + ng * 64],
                                      ps[0:NB, 0:ng * 64])

            def mixcl1(fc):
                f0 = fc * 1024
                fw = min(1024, GBLK * 64 - f0)
                ps = psB.tile([128, 1024], F32, tag="psB", name="ps1m")
                for g0 in range(0, fw, 512):
                    gw_ = min(512, fw - g0)
                    nc.tensor.matmul(ps[:, g0:g0 + gw_], mixMb1[:],
                                     B1[:, f0 + g0:f0 + g0 + gw_],
                                     start=True, stop=True)
                nc.scalar.activation(B1p[:, f0:f0 + fw], ps[0:NB, 0:fw],
                                     AF.Relu)

            for bk in range(4):
                tds1(bk)
                if bk > 0:
                    mixcl1(bk - 1)
            mixcl1(3)

            def mixds2(bk):
                npr = min(8, 26 - bk * 8)
                ps = psA.tile([128, 1024], F32, tag="psA", name="ps2")
                for g in range(npr):
                    gp = bk * 8 + g
                    nc.tensor.matmul(ps[:, g * 128:(g + 1) * 128],
                                     B1p[:, gp * 128:(gp + 1) * 128],
                                     mixM[:],
                                     start=True, stop=True)
                nc.vector.tensor_copy(
                    A2s[:, bk * 1024:bk * 1024 + npr * 128],
                    ps[:, 0:npr * 128])

            def tcl2(fc):
                f0 = fc * 1024
                fw = min(1024, 26 * 128 - f0)
                for par in range(2):
                    ps = psB.tile([128, 1024], F32, tag="psB", name="ps2t")
                    for g0 in range(0, fw, 512):
                        gw_ = min(512, fw - g0)
                        nc.tensor.matmul(
                            ps[:, g0:g0 + gw_],
                            w2d[par * 64:(par + 1) * 64, :],
                            A2s[par * 64:(par + 1) * 64,
                                f0 + g0:f0 + g0 + gw_],
                            start=True, stop=True)
                    if par == 0:
                        nc.scalar.activation(A2p[par][:, f0:f0 + fw],
                                             ps[:, 0:fw], AF.Relu,
                                             bias=b2[:, 0:1], scale=1.0)
                    else:
                        nc.vector.tensor_scalar(A2p[par][:, f0:f0 + fw],
                                                ps[:, 0:fw], b2[:, 0:1],
                                                0.0, ALU.add, ALU.max)

            for bk in range(4):
                mixds2(bk)
                if bk > 0:
                    tcl2(bk - 1)
            tcl2(3)

            if k > 0:
                emit_U(k - 1)

            def tds3(bk):
                ps = psA.tile([128, 1024], F32, tag="psA", name="ps3")
                for g in range(4):
                    gb = bk * 4 + g
                    nc.tensor.matmul(ps[:, g * 256:(g + 1) * 256],
                                     A2p[gb % 2][:, (gb // 2) * 128:
                                                 (gb // 2 + 1) * 128],
                                     w3[:],
                                     start=True, stop=True)
                nc.vector.tensor_copy(B3[0:NB, bk * 1024:(bk + 1) * 1024],
                                      ps[0:NB, :])

            def mixcl3(fc):
                f0 = fc * 1024
                ps = psB.tile([128, 1024], F32, tag="psB", name="ps3m")
                for g0 in (0, 512):
                    nc.tensor.matmul(ps[:, g0:g0 + 512], mixMb3[:],
                                     B3[0:NB + 1, f0 + g0:f0 + g0 + 512],
                                     start=True, stop=True)
                nc.scalar.activation(B3p[:, f0:f0 + 1024], ps[0:NB, :],
                                     AF.Relu)

            for bk in range(13):
                tds3(bk)
                if bk > 0:
                    mixcl3(bk - 1)
            mixcl3(12)

            def mixds4(bk, h):
                ng = min(8, 52 - bk * 8)
                ps = psA.tile([128, 1024], F32, tag="psA", name="ps4")
                for g in range(ng):
                    gb = bk * 8 + g
                    c0 = gb * 256 + h * 128
                    nc.tensor.matmul(ps[:, g * 128:(g + 1) * 128],
                                     B3p[:, c0:c0 + 128],
                                     mixM[:],
                                     start=True, stop=True)
                dst = A4[h][:, bk * 1024:bk * 1024 + ng * 128]
                if h == 0:
                    nc.scalar.copy(dst, ps[:, 0:ng * 128])
                else:
                    nc.vector.tensor_copy(dst, ps[:, 0:ng * 128])

            def tcl4(fc):
                f0 = fc * 1024
                fw = min(1024, CH_FREE - f0)
                for mt in range(2):
                    ps = psB.tile([128, 1024], F32, tag="psB", name="ps4t")
                    for g0 in range(0, fw, 512):
                        for kt in range(2):
                            nc.tensor.matmul(
                                ps[:, g0:g0 + 512],
                                w4k[kt][:, mt * 128:(mt + 1) * 128],
                                A4[kt][:, f0 + g0:f0 + g0 + 512],
                                start=(kt == 0), stop=(kt == 1))
                    if mt == 0:
                        nc.scalar.activation(X4[mt][:, f0:f0 + fw],
                                             ps[:, 0:fw], AF.Relu,
                                             bias=b4[:, mt:mt + 1],
                                             scale=1.0)
                    else:
                        nc.vector.tensor_scalar(X4[mt][:, f0:f0 + fw],
                                                ps[:, 0:fw],
                                                b4[:, mt:mt + 1], 0.0,
                                                ALU.add, ALU.max)

            for bk in range(7):
                mixds4(bk, 0)
                mixds4(bk, 1)
                if bk > 0:
                    tcl4(bk - 1)
            tcl4(6)

            # --- node-sum tree (24 nodes, stride 5 in blk) -> Fts chunk k
            for mt in range(2):
                xv = X4[mt][:].rearrange("p (gb blk) -> p gb blk", blk=128)
                nc.vector.tensor_tensor(xv[:, :, 0:60], xv[:, :, 0:60],
                                        xv[:, :, 60:120], ALU.add)
                nc.vector.tensor_tensor(xv[:, :, 0:30], xv[:, :, 0:30],
                                        xv[:, :, 30:60], ALU.add)
                nc.vector.tensor_tensor(xv[:, :, 0:15], xv[:, :, 0:15],
                                        xv[:, :, 15:30], ALU.add)
                nc.vector.tensor_tensor(xv[:, :, 0:5], xv[:, :, 0:5],
                                        xv[:, :, 5:10], ALU.add)
                dstv = Fts[mt][:, k * TP:(k + 1) * TP].rearrange(
                    "p (gb g5) -> p gb g5", g5=G5)
                nc.vector.tensor_tensor(dstv, xv[:, :, 0:5],
                                        xv[:, :, 10:15], ALU.add)

        emit_U(NCH - 1)

    # ===== LSTM-only constants (loaded after the GCN pools free SBUF) =====
    cons2 = ctx.enter_context(tc.tile_pool(name="cons2", bufs=1))
    lhf = load_ktiles(cons2, "lhf")
    lxb = load_ktiles(cons2, "lxb")

    def load_const2(name, shape, dt=F16):
        t = cons2.tile(shape, dt, name=name)
        nc.sync.dma_start(t[:], io[name][:])
        return t

    bgf = load_const2("bgf", [128, 8], F32)
    bgb = load_const2("bgb", [128, 8], F32)
    ident = load_const2("ident", [128, 128])
    wfct = []
    for qt in range(4):
        t = cons2.tile([128, 128], F16, name=f"wfct{qt}")
        nc.sync.dma_start(t[:], io["wfc"][qt * 128:(qt + 1) * 128, :])
        wfct.append(t)
    bfc = load_const2("bfc", [128, 1], F32)

    # ================= Phase 3: forward LSTM =================
    lp = ctx.enter_context(tc.tile_pool(name="lstm", bufs=1))
    Hf = lp.tile([128, 2 * ROWS], F16, name="Hf")
    Cf = lp.tile([128, 2 * ROWS], F16, name="Cf")
    nc.vector.memset(Hf[:], 0.0)
    nc.vector.memset(Cf[:], 0.0)
    gi = lp.tile([128, 2 * ROWS], F16, name="gi")
    gf = lp.tile([128, 2 * ROWS], F16, name="gf")
    go = lp.tile([128, 2 * ROWS], F16, name="go")
    tg = lp.tile([128, 2 * ROWS], F16, name="tg")
    tcl = lp.tile([128, 2 * ROWS], F16, name="tcl")
    tmp = lp.tile([128, 2 * ROWS], F16, name="tmp")
    gate_dst = [gi, gi, gf, gf, go, go, tg, tg]

    def hs(t, hh):
        # [128, (g:2, hh:2, 452)] -> the hh half across both 904-col groups
        return t[:].rearrange("p (g h r) -> p g h r", g=2,
                              r=HROWS)[:, :, hh, :]

    with tc.tile_pool(name="lps", bufs=4, space="PSUM") as ps_l, \
         tc.tile_pool(name="lpsb", bufs=2, space="PSUM") as ps_b:
        # two independent row-half recurrences, software-pipelined so the
        # activation/combine tail of one half hides under the other's matmuls
        for s in range(WIN):
            k0, par = s // 2, s % 2
            for hh in range(2):
                b0 = hh * (BL // 2)
                for mt in range(8):
                    ps = ps_l.tile([128, 512], F32, tag="lps", name="lps")
                    uv = Umt[mt][:].rearrange("p (b k two) -> p b k two",
                                              b=BL, two=2)
                    nc.tensor.matmul(
                        ps[:, 0:HROWS], ident[:],
                        uv[:, b0:b0 + BL // 2, k0:k0 + NW, par],
                        start=True, stop=False)
                    for kt in range(2):
                        nc.tensor.matmul(
                            ps[:, 0:HROWS],
                            lhf[kt][:, mt * 128:(mt + 1) * 128],
                            Hf[:, kt * ROWS + hh * HROWS:
                               kt * ROWS + (hh + 1) * HROWS],
                            start=False, stop=(kt == 1))
                    dst = gate_dst[mt][:, (mt % 2) * ROWS + hh * HROWS:
                                       (mt % 2) * ROWS + (hh + 1) * HROWS]
                    fn = AF.Sigmoid if mt < 6 else AF.Tanh
                    nc.scalar.activation(dst, ps[:, 0:HROWS], fn,
                                         bias=bgf[:, mt:mt + 1], scale=1.0)
                nc.vector.tensor_tensor(hs(tmp, hh), hs(gi, hh),
                                        hs(tg, hh), ALU.mult)
                nc.vector.tensor_tensor(hs(Cf, hh), hs(gf, hh),
                                        hs(Cf, hh), ALU.mult)
                nc.vector.tensor_tensor(hs(Cf, hh), hs(Cf, hh),
                                        hs(tmp, hh), ALU.add)
                nc.scalar.activation(hs(tcl, hh), hs(Cf, hh), AF.Tanh)
                nc.vector.tensor_tensor(hs(Hf, hh), hs(go, hh),
                                        hs(tcl, hh), ALU.mult)

        # ===== Phase 4: backward LSTM single step (only hb[:,0] used) =====
        Hb = lp.tile([128, 2 * ROWS], F16, name="Hb")
        kb = (WIN - 2) // 2
        for mt in [0, 1, 4, 5, 6, 7]:          # forget gate irrelevant (c0=0)
            ps = ps_b.tile([128, 1024], F32, tag="lpsb", name="lpsb")
            for hh in range(2):
                pslice = ps[:, hh * 512:hh * 512 + HROWS]
                b0 = hh * (BL // 2)
                for kt in range(2):
                    fv = Fts[kt][:].rearrange("p (b k two) -> p b k two",
                                              b=BL, two=2)
                    nc.tensor.matmul(
                        pslice, lxb[kt][:, mt * 128:(mt + 1) * 128],
                        fv[:, b0:b0 + BL // 2, kb:kb + NW, 1],
                        start=(kt == 0), stop=(kt == 1))
            dst = gate_dst[mt][:, (mt % 2) * ROWS:(mt % 2 + 1) * ROWS]
            dstv = dst.rearrange("p (h r) -> p h r", h=2)
            psv = ps[:].rearrange("p (h x) -> p h x", h=2)[:, :, 0:HROWS]
            fn = AF.Sigmoid if mt < 6 else AF.Tanh
            nc.scalar.activation(dstv, psv, fn,
                                 bias=bgb[:, mt:mt + 1], scale=1.0)
        nc.vector.tensor_tensor(tmp[:], gi[:], tg[:], ALU.mult)
        nc.scalar.activation(tcl[:], tmp[:], AF.Tanh)
        nc.vector.tensor_tensor(Hb[:], go[:], tcl[:], ALU.mult)

        # ===== Phase 5: FC head =====
        ps = ps_b.tile([128, 1024], F32, tag="lpsb", name="lpsf")
        rhs4 = [Hf[:, 0:ROWS], Hf[:, ROWS:2 * ROWS],
                Hb[:, 0:ROWS], Hb[:, ROWS:2 * ROWS]]
        for hh in range(2):
            for qt in range(4):
                nc.tensor.matmul(ps[:, hh * 512:hh * 512 + HROWS],
                                 wfct[qt][:],
                                 rhs4[qt].rearrange("p (h r) -> p h r",
                                                    h=2)[:, hh, :],
                                 start=(qt == 0), stop=(qt == 3))
        ob = lp.tile([EMB, ROWS], F32, name="ob")
        obv = ob[:].rearrange("p (h r) -> p h r", h=2)
        psv = ps[:].rearrange("p (h x) -> p h x", h=2)[:, :, 0:HROWS]
        nc.scalar.activation(obv, psv, AF.Identity,
                             bias=bfc[:, 0:1], scale=1.0)
        nc.sync.dma_start(io["out_d"][:], ob[:])

    ctx.close()


def _build_program():
    nc = bacc.Bacc("TRN2", target_bir_lowering=False, debug=False,
                   num_devices=NCORES)

    def din(name, shape, dt=F16):
        return nc.dram_tensor(name, shape, dt, kind="ExternalInput").ap()

    io = dict(
        x0A=din("x0A", [NCH, FIN, CH_FREE]),
        mixM=din("mixM", [NB, 128]),
        mixMb1=din("mixMb1", [NB + 1, 128]),
        mixMb3=din("mixMb3", [NB + 1, 128]),
        b1row=din("b1row", [1, GBLK * 64]),
        b3row=din("b3row", [1, GBLK * 256]),
        w1=din("w1", [FIN, 64]), w2d=din("w2d", [128, 128]),
        w3=din("w3", [128, 256]), w4=din("w4", [256, 256]),
        b2=din("b2", [128, 1], F32), b4=din("b4", [128, 2], F32),
        lxf=din("lxf", [256, 1024]), lhf=din("lhf", [256, 1024]),
        lxb=din("lxb", [256, 1024]),
        bgf=din("bgf", [128, 8], F32), bgb=din("bgb", [128, 8], F32),
        wfc=din("wfc", [512, 128]), bfc=din("bfc", [128, 1], F32),
        ident=din("ident", [128, 128]),
        out_d=nc.dram_tensor("out", [EMB, ROWS], F32,
                             kind="ExternalOutput").ap(),
    )
    with tile.TileContext(nc) as tc:
        _kernel_body(tc, io)
    nc.compile()
    return nc


def _host_prep(inputs):
    f16 = np.float16
    data = np.asarray(inputs["data"], np.float32)
    ei = np.asarray(inputs["edge_index"]).astype(np.int64)

    src = np.concatenate([ei[0], np.arange(N)])
    dst = np.concatenate([ei[1], np.arange(N)])
    deg = np.zeros(N, np.float32)
    np.add.at(deg, dst, 1.0)
    dinv = np.where(deg > 0, deg ** -0.5, 0.0).astype(np.float32)
    Ahat = np.zeros((N, N), np.float32)
    np.add.at(Ahat, (dst, src), dinv[src] * dinv[dst])
    mixM = np.kron(Ahat.T, np.eye(G5, dtype=np.float32)).astype(f16)  # [120,120]
    mixMp = np.zeros((NB, 128), f16)
    mixMp[:, 0:NB] = mixM

    def mixMb(bias_unused):
        m = np.zeros((NB + 1, 128), f16)
        m[0:NB, 0:NB] = mixM
        m[NB, 0:NB] = 1.0
        return m

    b1 = np.asarray(inputs["b1"], np.float32)
    b3 = np.asarray(inputs["b3"], np.float32)
    b1row = np.tile(b1[None, :], (GBLK, 1)).reshape(1, GBLK * 64).astype(f16)
    b3row = np.tile(b3[None, :], (GBLK, 1)).reshape(1, GBLK * 256).astype(f16)

    # x0A: [core][chunk b][c 6][gb*128 + blk], blk = n*5+g5, t = 5*gb+g5
    d = data.reshape(NCORES, BL, T, N, FIN)
    dpad = np.zeros((NCORES, BL, TP, N, FIN), np.float32)
    dpad[:, :, :T] = d
    dv = dpad.reshape(NCORES, BL, GBLK, G5, N, FIN)
    # -> [core, b, f, gb, n, g5]
    dv = dv.transpose(0, 1, 5, 2, 4, 3).reshape(NCORES, BL, FIN, GBLK, NB)
    x0A = np.zeros((NCORES, BL, FIN, GBLK, 128), np.float32)
    x0A[:, :, :, :, 0:NB] = dv
    x0A = np.ascontiguousarray(
        x0A.reshape(NCORES, BL, FIN, CH_FREE)).astype(f16)

    perm = np.concatenate([np.arange(0, H), np.arange(H, 2 * H),
                           np.arange(3 * H, 4 * H), np.arange(2 * H, 3 * H)])

    def prep_dir(wih, whh, bih, bhh):
        wihp = np.asarray(wih, np.float32)[perm] / N
        whhp = np.asarray(whh, np.float32)[perm]
        bg = (np.asarray(bih, np.float32) + np.asarray(bhh, np.float32))[perm]
        return (np.ascontiguousarray(wihp.T).astype(f16),
                np.ascontiguousarray(whhp.T).astype(f16),
                np.ascontiguousarray(bg.reshape(8, 128).T).astype(np.float32))

    lxf, lhf, bgf = prep_dir(inputs["lstm_Wih_f"], inputs["lstm_Whh_f"],
                             inputs["lstm_bih_f"], inputs["lstm_bhh_f"])
    lxb, _lhb, bgb = prep_dir(inputs["lstm_Wih_b"], inputs["lstm_Whh_b"],
                              inputs["lstm_bih_b"], inputs["lstm_bhh_b"])

    com = {
        "mixM": mixMp,
        "mixMb1": mixMb(None),
        "mixMb3": mixMb(None),
        "b1row": b1row,
        "b3row": b3row,
        "w1": np.asarray(inputs["W1"], np.float32).astype(f16),
        "w2d": np.concatenate([np.asarray(inputs["W2"], np.float32)] * 2,
                              axis=0).astype(f16),
        "w3": np.asarray(inputs["W3"], np.float32).astype(f16),
        "w4": np.asarray(inputs["W4"], np.float32).astype(f16),
        "b2": np.asarray(inputs["b2"], np.float32).reshape(128, 1),
        "b4": np.ascontiguousarray(
            np.asarray(inputs["b4"], np.float32).reshape(2, 128).T),
        "lxf": lxf, "lhf": lhf, "lxb": lxb, "bgf": bgf, "bgb": bgb,
        "wfc": np.asarray(inputs["Wfc"], np.float32).astype(f16),
        "bfc": np.asarray(inputs["bfc"], np.float32).reshape(128, 1),
        "ident": np.eye(128, dtype=f16),
    }
    return [dict(com, x0A=x0A[c]) for c in range(NCORES)]


TRACE = False          # set by test harness to capture an NTFF profile


def kernel(**inputs) -> np.ndarray:
    if "nc" not in _CACHE:
        _CACHE["nc"] = _build_program()
    nc = _CACHE["nc"]
    in_maps = _host_prep(inputs)
    res = bass_utils.run_bass_kernel_spmd(nc, in_maps,
                                          core_ids=list(range(NCORES)),
                                          trace=TRACE)
    _CACHE["last_res"] = res
    outs = []
    for c in range(NCORES):
        o = res.results[c]["out"]                       # [128, 904]
        outs.append(o.reshape(EMB, BL, NW).transpose(1, 2, 0))
    return np.concatenate(outs, 0).astype(np.float32)   # [64, 113, 128]


if __name__ == "__main__":
    import reference
    ins = {k: np.asarray(v) for k, v in reference.setup_inputs().items()}
    out = kernel(**ins)
    print("kernel out", out.shape, out.dtype, float(np.abs(out).max()))
